# revision 1
# baseline (speedup 1.0000x reference)
# HGNNP hypergraph convolution on 8 Trainium2 NeuronCores (Bass/Tile).
#
# Reference computation:
#   H      = relu(X @ W.T + b)                    [N, 128]
#   e_feat = segment_mean(H[v_idx], e_idx, E)     [E, 128]
#   out    = relu(segment_mean(e_feat[e_idx], v_idx, N))
#
# Strategy (vertex sharding, one kernel launch, in-kernel AllReduce):
#   * Each core owns a contiguous vertex range (VPC rows of X) and computes
#     its H shard with TensorE (X^T is pre-transposed on the host so the
#     contraction dim lands on partitions).
#   * Incidence entries are routed to the core owning their vertex.  Within a
#     core they are bucketed by edge-block (128 edges) and padded to a fixed
#     number of 128-entry tiles per block.  A dma_gather pulls H rows for each
#     entry; a per-tile 0/1 selection matrix (is_equal vs an iota row) and a
#     PSUM-accumulated matmul reduce the tile into the block's 128 edge rows.
#   * Per-core partial edge sums are AllReduced, scaled by 1/edge_degree, cast
#     to fp16 -> e_feat table.
#   * Phase B mirrors phase A keyed by vertex: gather e_feat rows per entry,
#     selection-matmul into 128-vertex blocks, scale by 1/vertex_degree, relu,
#     write the core's output rows.
#   * Gather payloads are fp16 (halves the dominant memory traffic);
#     accumulation stays fp32 in PSUM.
import os
from dataclasses import dataclass

import numpy as np

P = 128


def chunk_of(n):
    """Largest divisor of n that is a multiple of 128 and <= 1024 idxs
    (<= 64 descriptors per SDMA engine keeps single_packet legal)."""
    for ck in range(768, 127, -128):
        if n % ck == 0:
            return ck
    return 128


@dataclass(frozen=True)
class Cfg:
    n_cores: int
    in_ch: int
    out_ch: int
    vpc: int        # vertices per core (multiple of 128)
    ne_pad: int     # padded edge count (multiple of 128)
    ta: int         # 128-entry tiles per (core, edge-block)
    tv: int         # 128-entry tiles per (core, vertex-block)
    gb_a: int       # edge-blocks per gather in phase A
    gb_b: int       # vertex-blocks per gather in phase B

    @property
    def eb(self):
        return self.ne_pad // P

    @property
    def vb(self):
        return self.vpc // P

    @property
    def na(self):
        return self.eb * self.ta * P

    @property
    def nb(self):
        return self.vb * self.tv * P


# Real problem dimensions.
N_VERTICES = 100000
N_EDGES = 25000
NNZ = 3200000
NV_PAD = 100352           # 8 * 12544
REAL = dict(n_cores=8, in_ch=256, out_ch=128, vpc=12544, ne_pad=25088,
            gb_a=4, gb_b=2)

_PROG_CACHE = {}
LAST_RESULTS = None       # BassKernelResults of the most recent run (for test.py)


def build_program(cfg: Cfg):
    """Emit the SPMD Bass program (identical on all cores; per-core behavior
    comes entirely from per-core input tensors)."""
    import concourse.bass as bass
    import concourse.mybir as mybir
    import concourse.tile as tile
    from concourse import bacc

    dt = mybir.dt
    OC = cfg.out_ch
    assert cfg.in_ch % P == 0
    KC = cfg.in_ch // P

    nc = bacc.Bacc("TRN2", target_bir_lowering=False, debug=False,
                   num_devices=cfg.n_cores)

    # ---- I/O ----
    xt = nc.dram_tensor("xt", [cfg.in_ch, cfg.vpc], dt.float16, kind="ExternalInput")
    wt = nc.dram_tensor("wt", [cfg.in_ch, OC], dt.float16, kind="ExternalInput")
    bmat = nc.dram_tensor("bmat", [P, OC], dt.float32, kind="ExternalInput")
    iota = nc.dram_tensor("iota", [P, P], dt.float16, kind="ExternalInput")
    idxa = nc.dram_tensor("idxa", [P, cfg.na // 16], dt.int16, kind="ExternalInput")
    eloc = nc.dram_tensor("eloc", [P, cfg.na // P], dt.float16, kind="ExternalInput")
    idxb = nc.dram_tensor("idxb", [P, cfg.nb // 16], dt.int16, kind="ExternalInput")
    vloc = nc.dram_tensor("vloc", [P, cfg.nb // P], dt.float16, kind="ExternalInput")
    re_p = nc.dram_tensor("re", [P, cfg.eb], dt.float32, kind="ExternalInput")
    rv_p = nc.dram_tensor("rv", [P, cfg.vb], dt.float32, kind="ExternalInput")
    out = nc.dram_tensor("out", [cfg.vpc, OC], dt.float32, kind="ExternalOutput")

    # ---- internal DRAM ----
    hdr = nc.dram_tensor("hdram", [cfg.vpc + P, OC], dt.float16)
    esum = nc.dram_tensor("esum", [cfg.ne_pad, OC], dt.float16)
    esum_red = nc.dram_tensor("esum_red", [cfg.ne_pad, OC], dt.float16,
                              addr_space="Shared")
    efeat = nc.dram_tensor("efeat", [cfg.ne_pad + P, OC], dt.float16)

    def bcast_free(ap2d, n):
        # [P, C] -> [P, C, n] with the trailing dim broadcast
        return bass.AP(tensor=ap2d.tensor, offset=ap2d.offset,
                       ap=[*ap2d.ap, [0, n]])

    def bcast_mid(ap2d, n):
        # [P, C] -> [P, n, C] with the middle dim broadcast
        return bass.AP(tensor=ap2d.tensor, offset=ap2d.offset,
                       ap=[ap2d.ap[0], [0, n], ap2d.ap[1]])

    with tile.TileContext(nc) as tc:
        import contextlib
        with contextlib.ExitStack() as ctx:
            const = ctx.enter_context(tc.tile_pool(name="const", bufs=1))
            work = ctx.enter_context(tc.tile_pool(name="work", bufs=3))
            gpool = ctx.enter_context(tc.tile_pool(name="gpool", bufs=2))
            ipool = ctx.enter_context(tc.tile_pool(name="ipool", bufs=2))
            spool = ctx.enter_context(tc.tile_pool(name="spool", bufs=2))
            psum = ctx.enter_context(tc.tile_pool(name="psum", bufs=4, space="PSUM"))

            # ---- constants ----
            xt_sb = const.tile([P, KC, cfg.vpc], dt.float16)
            for k in range(KC):
                nc.sync.dma_start(out=xt_sb[:, k, :], in_=xt[k * P:(k + 1) * P, :])
            wt_sb = const.tile([P, KC, OC], dt.float16)
            for k in range(KC):
                nc.sync.dma_start(out=wt_sb[:, k, :], in_=wt[k * P:(k + 1) * P, :])
            bb = const.tile([P, OC], dt.float32)
            nc.sync.dma_start(out=bb[:], in_=bmat[:, :])
            iota_sb = const.tile([P, P], dt.float16)
            nc.sync.dma_start(out=iota_sb[:], in_=iota[:, :])
            eloc_sb = const.tile([P, cfg.na // P], dt.float16)
            nc.sync.dma_start(out=eloc_sb[:], in_=eloc[:, :])
            vloc_sb = const.tile([P, cfg.nb // P], dt.float16)
            nc.sync.dma_start(out=vloc_sb[:], in_=vloc[:, :])
            re_sb = const.tile([P, cfg.eb], dt.float32)
            nc.sync.dma_start(out=re_sb[:], in_=re_p[:, :])
            rv_sb = const.tile([P, cfg.vb], dt.float32)
            nc.sync.dma_start(out=rv_sb[:], in_=rv_p[:, :])

            # ---- stage H: H = relu(X @ W.T + b) -> fp16 rows in DRAM ----
            for vt in range(cfg.vb):
                ps = psum.tile([P, OC], dt.float32, space="PSUM")
                for k in range(KC):
                    nc.tensor.matmul(out=ps[:],
                                     lhsT=xt_sb[:, k, vt * P:(vt + 1) * P],
                                     rhs=wt_sb[:, k, :],
                                     start=(k == 0), stop=(k == KC - 1))
                tmp = work.tile([P, OC], dt.float32)
                nc.vector.tensor_add(out=tmp[:], in0=ps[:], in1=bb[:])
                h_t = work.tile([P, OC], dt.float16)
                nc.vector.tensor_scalar_max(out=h_t[:], in0=tmp[:], scalar1=0.0)
                nc.sync.dma_start(out=hdr[vt * P:(vt + 1) * P, :], in_=h_t[:])
            zt = work.tile([P, OC], dt.float16)
            nc.vector.memset(zt[:], 0.0)
            nc.sync.dma_start(out=hdr[cfg.vpc:cfg.vpc + P, :], in_=zt[:])

            # ---- phase A: partial edge sums ----
            n_ga = cfg.gb_a * cfg.ta * P          # idxs per gather
            assert cfg.eb % cfg.gb_a == 0
            for g in range(cfg.eb // cfg.gb_a):
                ixt = ipool.tile([P, n_ga // 16], dt.int16)
                nc.sync.dma_start(out=ixt[:],
                                  in_=idxa[:, g * (n_ga // 16):(g + 1) * (n_ga // 16)])
                gt = gpool.tile([P, cfg.gb_a * cfg.ta, OC], dt.float16)
                ck = chunk_of(n_ga)
                for q in range(n_ga // ck):
                    nc.gpsimd.dma_gather(
                        gt[:, q * (ck // P):(q + 1) * (ck // P), :], hdr[:, :],
                        ixt[:, q * (ck // 16):(q + 1) * (ck // 16)],
                        ck, ck, OC, single_packet=True)
                for j in range(cfg.gb_a):
                    ebi = g * cfg.gb_a + j
                    s_t = spool.tile([P, cfg.ta, P], dt.float16)
                    nc.vector.tensor_tensor(
                        out=s_t[:],
                        in0=bcast_free(eloc_sb[:, ebi * cfg.ta:(ebi + 1) * cfg.ta], P),
                        in1=bcast_mid(iota_sb[:, :], cfg.ta),
                        op=mybir.AluOpType.is_equal)
                    ps = psum.tile([P, OC], dt.float32, space="PSUM")
                    for t in range(cfg.ta):
                        nc.tensor.matmul(out=ps[:], lhsT=s_t[:, t, :],
                                         rhs=gt[:, j * cfg.ta + t, :],
                                         start=(t == 0), stop=(t == cfg.ta - 1))
                    es = work.tile([P, OC], dt.float16)
                    nc.vector.tensor_copy(out=es[:], in_=ps[:])
                    nc.sync.dma_start(out=esum[ebi * P:(ebi + 1) * P, :], in_=es[:])

            # ---- AllReduce partial edge sums ----
            nc.gpsimd.collective_compute(
                "AllReduce", mybir.AluOpType.add,
                replica_groups=[list(range(cfg.n_cores))],
                ins=[esum.ap().opt()], outs=[esum_red.ap().opt()])

            # ---- e_feat = esum_red * (1/edge_deg) -> fp16 table ----
            for et in range(cfg.eb):
                t_in = work.tile([P, OC], dt.float16)
                nc.sync.dma_start(out=t_in[:], in_=esum_red[et * P:(et + 1) * P, :])
                ef = work.tile([P, OC], dt.float16)
                nc.vector.tensor_scalar_mul(out=ef[:], in0=t_in[:],
                                            scalar1=re_sb[:, et:et + 1])
                nc.sync.dma_start(out=efeat[et * P:(et + 1) * P, :], in_=ef[:])
            ztb = work.tile([P, OC], dt.float16)
            nc.vector.memset(ztb[:], 0.0)
            nc.sync.dma_start(out=efeat[cfg.ne_pad:cfg.ne_pad + P, :], in_=ztb[:])

            # ---- phase B: vertex means + relu ----
            n_gb = cfg.gb_b * cfg.tv * P
            assert cfg.vb % cfg.gb_b == 0
            for g in range(cfg.vb // cfg.gb_b):
                ixt = ipool.tile([P, n_gb // 16], dt.int16)
                nc.sync.dma_start(out=ixt[:],
                                  in_=idxb[:, g * (n_gb // 16):(g + 1) * (n_gb // 16)])
                gt = gpool.tile([P, cfg.gb_b * cfg.tv, OC], dt.float16)
                ck = chunk_of(n_gb)
                for q in range(n_gb // ck):
                    nc.gpsimd.dma_gather(
                        gt[:, q * (ck // P):(q + 1) * (ck // P), :], efeat[:, :],
                        ixt[:, q * (ck // 16):(q + 1) * (ck // 16)],
                        ck, ck, OC, single_packet=True)
                for j in range(cfg.gb_b):
                    vbi = g * cfg.gb_b + j
                    s_t = spool.tile([P, cfg.tv, P], dt.float16)
                    nc.vector.tensor_tensor(
                        out=s_t[:],
                        in0=bcast_free(vloc_sb[:, vbi * cfg.tv:(vbi + 1) * cfg.tv], P),
                        in1=bcast_mid(iota_sb[:, :], cfg.tv),
                        op=mybir.AluOpType.is_equal)
                    ps = psum.tile([P, OC], dt.float32, space="PSUM")
                    for t in range(cfg.tv):
                        nc.tensor.matmul(out=ps[:], lhsT=s_t[:, t, :],
                                         rhs=gt[:, j * cfg.tv + t, :],
                                         start=(t == 0), stop=(t == cfg.tv - 1))
                    ot = work.tile([P, OC], dt.float32)
                    nc.vector.tensor_scalar(out=ot[:], in0=ps[:],
                                            scalar1=rv_sb[:, vbi:vbi + 1],
                                            scalar2=0.0,
                                            op0=mybir.AluOpType.mult,
                                            op1=mybir.AluOpType.max)
                    nc.sync.dma_start(out=out[vbi * P:(vbi + 1) * P, :], in_=ot[:])

    nc.compile()
    return nc


def pack_inputs(cfg: Cfg, X, W, b, v_idx, e_idx):
    """Host-side preprocessing: shard by vertex range, bucket entries, pad,
    and build the per-core input dicts."""
    f16, f32, i16 = np.float16, np.float32, np.int16
    C, VPC, EB, VB, TA, TV = cfg.n_cores, cfg.vpc, cfg.eb, cfg.vb, cfg.ta, cfg.tv
    NA, NB = cfg.na, cfg.nb
    nv_pad = C * VPC
    n_edges = int(e_idx.max()) + 1 if len(e_idx) else 0

    v = np.asarray(v_idx).astype(np.int64)
    e = np.asarray(e_idx).astype(np.int64)
    core = v // VPC

    # ----- phase A routing: bucket by (core, edge-block), any order inside -----
    blk = core * EB + e // P
    order = np.argsort(blk, kind="stable")
    cnt = np.bincount(blk, minlength=C * EB)
    assert cnt.max() <= TA * P, f"phase A padding overflow: {cnt.max()} > {TA * P}"
    starts = np.zeros(C * EB, np.int64)
    np.cumsum(cnt[:-1], out=starts[1:])
    ofs = np.arange(len(v), dtype=np.int64) - np.repeat(starts, cnt)
    blk_s = blk[order]
    core_s = blk_s // EB
    dest = core_s * NA + (blk_s % EB) * (TA * P) + ofs
    idxa_all = np.full(C * NA, VPC, i16)
    idxa_all[dest] = (v[order] - core_s * VPC).astype(i16)
    eloc_all = np.zeros(C * NA, f16)
    eloc_all[dest] = (e[order] % P).astype(f16)

    # ----- phase B routing: bucket by vertex-block -----
    blkb = v // P                      # == core * VB + local block
    order_b = np.argsort(blkb, kind="stable")
    cntb = np.bincount(blkb, minlength=C * VB)
    assert cntb.max() <= TV * P, f"phase B padding overflow: {cntb.max()} > {TV * P}"
    starts_b = np.zeros(C * VB, np.int64)
    np.cumsum(cntb[:-1], out=starts_b[1:])
    ofs_b = np.arange(len(v), dtype=np.int64) - np.repeat(starts_b, cntb)
    blkb_s = blkb[order_b]
    core_b = blkb_s // VB
    dest_b = core_b * NB + (blkb_s % VB) * (TV * P) + ofs_b
    idxb_all = np.full(C * NB, cfg.ne_pad, i16)
    idxb_all[dest_b] = e[order_b].astype(i16)
    vloc_all = np.zeros(C * NB, f16)
    vloc_all[dest_b] = (v[order_b] % P).astype(f16)

    # ----- degrees -----
    edeg = np.bincount(e, minlength=cfg.ne_pad).astype(f32)
    re = (1.0 / np.maximum(edeg, 1.0)).astype(f32)
    re_p = np.ascontiguousarray(re.reshape(EB, P).T)
    vdeg = np.bincount(v, minlength=nv_pad).astype(f32)
    rv = (1.0 / np.maximum(vdeg, 1.0)).astype(f32)

    # ----- dense inputs -----
    nv = X.shape[0]
    xt_full = np.zeros((cfg.in_ch, nv_pad), f16)
    xt_full[:, :nv] = np.asarray(X, np.float32).T.astype(f16)
    wt = np.ascontiguousarray(np.asarray(W, np.float32).T.astype(f16))
    bmat = np.tile(np.asarray(b, f32)[None, :], (P, 1))
    iota = np.tile(np.arange(P, dtype=f16)[None, :], (P, 1))

    def wrap16(a):
        # gather index layout: idx i -> [16 partitions, i // 16], replicated x8
        return np.ascontiguousarray(np.tile(a.reshape(-1, 16).T, (P // 16, 1)))

    def pack128(a):
        # per-tile column layout: entry i -> [i % 128, i // 128]
        return np.ascontiguousarray(a.reshape(-1, P).T)

    in_maps = []
    for c in range(C):
        in_maps.append({
            "xt": np.ascontiguousarray(xt_full[:, c * VPC:(c + 1) * VPC]),
            "wt": wt,
            "bmat": bmat,
            "iota": iota,
            "idxa": wrap16(idxa_all[c * NA:(c + 1) * NA]),
            "eloc": pack128(eloc_all[c * NA:(c + 1) * NA]),
            "idxb": wrap16(idxb_all[c * NB:(c + 1) * NB]),
            "vloc": pack128(vloc_all[c * NB:(c + 1) * NB]),
            "re": re_p,
            "rv": np.ascontiguousarray(rv[c * VPC:(c + 1) * VPC].reshape(VB, P).T),
        })
    return in_maps


def make_cfg(v_idx, e_idx, base=REAL):
    """Padding tile counts depend on the data; compute them here so the same
    builder serves any input of the real shapes."""
    v = np.asarray(v_idx).astype(np.int64)
    e = np.asarray(e_idx).astype(np.int64)
    eb = base["ne_pad"] // P
    vb = base["vpc"] // P
    blk = (v // base["vpc"]) * eb + e // P
    ta = int(np.ceil(np.bincount(blk, minlength=base["n_cores"] * eb).max() / P))
    blkb = v // P
    tv = int(np.ceil(np.bincount(blkb, minlength=base["n_cores"] * vb).max() / P))
    return Cfg(ta=max(ta, 1), tv=max(tv, 1), **base)


def run(cfg: Cfg, in_maps, trace=False):
    global LAST_RESULTS
    from concourse.bass_utils import run_bass_kernel_spmd
    key = (cfg.ta, cfg.tv)
    if key not in _PROG_CACHE:
        _PROG_CACHE[key] = build_program(cfg)
    nc = _PROG_CACHE[key]
    res = run_bass_kernel_spmd(nc, in_maps, core_ids=list(range(cfg.n_cores)),
                               trace=trace)
    LAST_RESULTS = res
    return res


def kernel(X, W, b, v_idx, e_idx, trace=False):
    cfg = make_cfg(v_idx, e_idx)
    in_maps = pack_inputs(cfg, X, W, b, v_idx, e_idx)
    res = run(cfg, in_maps, trace=trace)
    out = np.concatenate([res.results[c]["out"] for c in range(cfg.n_cores)], axis=0)
    return np.ascontiguousarray(out[:N_VERTICES]).astype(np.float32)



# revision 11
# speedup vs baseline: 1.2481x; 1.2481x over previous
# HGNNP hypergraph convolution on 8 Trainium2 NeuronCores (Bass/Tile).
#
# Reference computation:
#   H      = relu(X @ W.T + b)                    [N, 128]
#   e_feat = segment_mean(H[v_idx], e_idx, E)     [E, 128]
#   out    = relu(segment_mean(e_feat[e_idx], v_idx, N))
#
# Strategy (vertex sharding, one kernel launch, in-kernel AllReduce):
#   * Each core owns a contiguous vertex range (VPC rows of X) and computes
#     its H shard with TensorE (X^T is pre-transposed on the host so the
#     contraction dim lands on partitions).
#   * Incidence entries are routed to the core owning their vertex.  Within a
#     core they are bucketed by edge-block (128 edges), sorted by target row,
#     and padded per block to a per-block tile budget (max over cores, so the
#     SPMD program is identical on every core).  A dma_gather pulls H rows
#     for each entry; a per-tile 0/1 selection matrix (is_equal vs an iota
#     row) and a PSUM-accumulated matmul reduce the tile into the block's
#     128 edge rows.
#   * Per-core partial edge sums are AllReduced in 4 slices (the first
#     slices overlap phase A's tail), scaled by 1/edge_degree -> fp16
#     e_feat table.
#   * Phase B mirrors phase A keyed by vertex: gather e_feat rows per entry,
#     selection-matmul into 128-vertex blocks, scale by 1/vertex_degree,
#     relu, write the core's output rows.
#   * Gather payloads are fp16 (256 B/descriptor, the dma_gather minimum);
#     accumulation stays fp32 in PSUM.  The gathers are the only significant
#     device cost (~8 ns/descriptor); per-block budgets trim the padded
#     descriptor count vs a global uniform budget.
from dataclasses import dataclass

import numpy as np

P = 128


def chunks_of(n, cap=9216):
    """Split n idxs into as few chunks as possible, each %128==0 and <=cap.
    Chunk size / single_packet measured performance-neutral on HW from 768
    to 9216 idx/call; big chunks keep Pool-engine desc-gen overhead low."""
    k = -(-n // cap)
    per = -(-n // (k * 128)) * 128
    out = []
    left = n
    while left > 0:
        c = min(per, left)
        out.append(c)
        left -= c
    assert sum(out) == n and all(c % 128 == 0 for c in out)
    return out


@dataclass(frozen=True)
class Cfg:
    n_cores: int
    in_ch: int
    out_ch: int
    vpc: int            # vertices per core (multiple of 128)
    ne_pad: int         # padded edge count (multiple of 128)
    ba: tuple           # per-edge-block tile budgets (len eb)
    bb: tuple           # per-vertex-block tile budgets (len vb)
    gb_a: int           # edge-blocks per gather group in phase A
    gb_b: int           # vertex-blocks per gather group in phase B
    gcap: int = 9216    # max idx per dma_gather call
    gsp: bool = False   # single_packet flag for gather calls

    @property
    def eb(self):
        return self.ne_pad // P

    @property
    def vb(self):
        return self.vpc // P

    @property
    def na(self):
        return sum(self.ba) * P

    @property
    def nb(self):
        return sum(self.bb) * P


# Real problem dimensions.
N_VERTICES = 100000
N_EDGES = 25000
NNZ = 3200000
NV_PAD = 100352           # 8 * 12544
REAL = dict(n_cores=8, in_ch=256, out_ch=128, vpc=12544, ne_pad=25088,
            gb_a=7, gb_b=2)

_PROG_CACHE = {}
LAST_RESULTS = None       # BassKernelResults of the most recent run (for test.py)


def build_program(cfg: Cfg, ablate=frozenset()):
    """Emit the SPMD Bass program (identical on all cores; per-core behavior
    comes entirely from per-core input tensors).

    ablate is a profiling-only hook (see profile_variants.py): "coll" drops
    the AllReduce, "ga"/"gb" replace the phase A/B gathers with memsets.
    The graded path always uses the default empty set."""
    import concourse.bass as bass
    import concourse.mybir as mybir
    import concourse.tile as tile
    from concourse import bacc

    dt = mybir.dt
    OC = cfg.out_ch
    assert cfg.in_ch % P == 0
    KC = cfg.in_ch // P

    # per-block tile offsets (tiles, within a core's stream)
    off_a = np.concatenate([[0], np.cumsum(cfg.ba)]).astype(int)
    off_b = np.concatenate([[0], np.cumsum(cfg.bb)]).astype(int)

    nc = bacc.Bacc("TRN2", target_bir_lowering=False, debug=False,
                   num_devices=cfg.n_cores)

    # ---- I/O ----
    xt = nc.dram_tensor("xt", [cfg.in_ch, cfg.vpc], dt.float16, kind="ExternalInput")
    wt = nc.dram_tensor("wt", [cfg.in_ch, OC], dt.float16, kind="ExternalInput")
    bmat = nc.dram_tensor("bmat", [P, OC], dt.float32, kind="ExternalInput")
    iota = nc.dram_tensor("iota", [P, P], dt.float16, kind="ExternalInput")
    idxa = nc.dram_tensor("idxa", [P, cfg.na // 16], dt.int16, kind="ExternalInput")
    eloc = nc.dram_tensor("eloc", [P, cfg.na // P], dt.float16, kind="ExternalInput")
    idxb = nc.dram_tensor("idxb", [P, cfg.nb // 16], dt.int16, kind="ExternalInput")
    vloc = nc.dram_tensor("vloc", [P, cfg.nb // P], dt.float16, kind="ExternalInput")
    re_p = nc.dram_tensor("re", [P, cfg.eb], dt.float32, kind="ExternalInput")
    rv_p = nc.dram_tensor("rv", [P, cfg.vb], dt.float32, kind="ExternalInput")
    out = nc.dram_tensor("out", [cfg.vpc, OC], dt.float32, kind="ExternalOutput")

    # ---- internal DRAM ----
    hdr = nc.dram_tensor("hdram", [cfg.vpc + P, OC], dt.float16)
    esum = nc.dram_tensor("esum", [cfg.ne_pad, OC], dt.float16)
    esum_red = nc.dram_tensor("esum_red", [cfg.ne_pad, OC], dt.float16,
                              addr_space="Shared")
    efeat = nc.dram_tensor("efeat", [cfg.ne_pad + P, OC], dt.float16)

    def bcast_free(ap2d, n):
        # [P, C] -> [P, C, n] with the trailing dim broadcast
        return bass.AP(tensor=ap2d.tensor, offset=ap2d.offset,
                       ap=[*ap2d.ap, [0, n]])

    def bcast_mid(ap2d, n):
        # [P, C] -> [P, n, C] with the middle dim broadcast
        return bass.AP(tensor=ap2d.tensor, offset=ap2d.offset,
                       ap=[ap2d.ap[0], [0, n], ap2d.ap[1]])

    max_tg_a = max(sum(cfg.ba[g:g + cfg.gb_a])
                   for g in range(0, cfg.eb, cfg.gb_a))
    max_tg_b = max(sum(cfg.bb[g:g + cfg.gb_b])
                   for g in range(0, cfg.vb, cfg.gb_b))
    max_tg = max(max_tg_a, max_tg_b)
    max_w = max(max(cfg.ba), max(cfg.bb))

    with tile.TileContext(nc) as tc:
        import contextlib
        with contextlib.ExitStack() as ctx:
            const = ctx.enter_context(tc.tile_pool(name="const", bufs=1))
            work = ctx.enter_context(tc.tile_pool(name="work", bufs=3))
            gpool = ctx.enter_context(tc.tile_pool(name="gpool", bufs=2))
            ipool = ctx.enter_context(tc.tile_pool(name="ipool", bufs=2))
            spool = ctx.enter_context(tc.tile_pool(name="spool", bufs=2))
            psum = ctx.enter_context(tc.tile_pool(name="psum", bufs=4, space="PSUM"))

            # ---- constants ----
            xt_sb = const.tile([P, KC, cfg.vpc], dt.float16)
            for k in range(KC):
                nc.sync.dma_start(out=xt_sb[:, k, :], in_=xt[k * P:(k + 1) * P, :])
            wt_sb = const.tile([P, KC, OC], dt.float16)
            for k in range(KC):
                nc.sync.dma_start(out=wt_sb[:, k, :], in_=wt[k * P:(k + 1) * P, :])
            bb_t = const.tile([P, OC], dt.float32)
            nc.sync.dma_start(out=bb_t[:], in_=bmat[:, :])
            iota_sb = const.tile([P, P], dt.float16)
            nc.sync.dma_start(out=iota_sb[:], in_=iota[:, :])
            eloc_sb = const.tile([P, cfg.na // P], dt.float16)
            nc.sync.dma_start(out=eloc_sb[:], in_=eloc[:, :])
            vloc_sb = const.tile([P, cfg.nb // P], dt.float16)
            nc.sync.dma_start(out=vloc_sb[:], in_=vloc[:, :])
            re_sb = const.tile([P, cfg.eb], dt.float32)
            nc.sync.dma_start(out=re_sb[:], in_=re_p[:, :])
            rv_sb = const.tile([P, cfg.vb], dt.float32)
            nc.sync.dma_start(out=rv_sb[:], in_=rv_p[:, :])

            # ---- stage H: H = relu(X @ W.T + b) -> fp16 rows in DRAM ----
            for vt in range(cfg.vb):
                ps = psum.tile([P, OC], dt.float32, space="PSUM")
                for k in range(KC):
                    nc.tensor.matmul(out=ps[:],
                                     lhsT=xt_sb[:, k, vt * P:(vt + 1) * P],
                                     rhs=wt_sb[:, k, :],
                                     start=(k == 0), stop=(k == KC - 1))
                tmp = work.tile([P, OC], dt.float32)
                nc.vector.tensor_add(out=tmp[:], in0=ps[:], in1=bb_t[:])
                h_t = work.tile([P, OC], dt.float16)
                nc.vector.tensor_scalar_max(out=h_t[:], in0=tmp[:], scalar1=0.0)
                nc.sync.dma_start(out=hdr[vt * P:(vt + 1) * P, :], in_=h_t[:])
            zt = work.tile([P, OC], dt.float16)
            nc.vector.memset(zt[:], 0.0)
            nc.sync.dma_start(out=hdr[cfg.vpc:cfg.vpc + P, :], in_=zt[:])

            # ---- phase A: partial edge sums ----
            for g0 in range(0, cfg.eb, cfg.gb_a):
                js = list(range(g0, min(g0 + cfg.gb_a, cfg.eb)))
                t_base = int(off_a[g0])
                tg = int(off_a[js[-1] + 1]) - t_base
                ixt = ipool.tile([P, max_tg * 8], dt.int16)
                nc.sync.dma_start(
                    out=ixt[:, :tg * 8],
                    in_=idxa[:, t_base * 8:(t_base + tg) * 8])
                gt = gpool.tile([P, max_tg, OC], dt.float16)
                o = 0
                if "ga" in ablate:
                    nc.vector.memset(gt[:], 0.0)
                else:
                    for ck in chunks_of(tg * P, cfg.gcap):
                        nc.gpsimd.dma_gather(
                            gt[:, o // P:(o + ck) // P, :], hdr[:, :],
                            ixt[:, o // 16:(o + ck) // 16],
                            ck, ck, OC, single_packet=cfg.gsp)
                        o += ck
                for j in js:
                    w = cfg.ba[j]
                    s = int(off_a[j]) - t_base
                    s_t = spool.tile([P, max_w, P], dt.float16)
                    nc.vector.tensor_tensor(
                        out=s_t[:, :w, :],
                        in0=bcast_free(eloc_sb[:, int(off_a[j]):int(off_a[j]) + w], P),
                        in1=bcast_mid(iota_sb[:, :], w),
                        op=mybir.AluOpType.is_equal)
                    ps = psum.tile([P, OC], dt.float32, space="PSUM")
                    for t in range(w):
                        nc.tensor.matmul(out=ps[:], lhsT=s_t[:, t, :],
                                         rhs=gt[:, s + t, :],
                                         start=(t == 0), stop=(t == w - 1))
                    es = work.tile([P, OC], dt.float16)
                    nc.vector.tensor_copy(out=es[:], in_=ps[:])
                    nc.sync.dma_start(out=esum[j * P:(j + 1) * P, :], in_=es[:])

            # ---- AllReduce partial edge sums (sliced: overlaps phase A) ----
            NSL = 4
            assert cfg.eb % NSL == 0
            sl_rows = (cfg.eb // NSL) * P
            if "coll" not in ablate:
                for s in range(NSL):
                    nc.gpsimd.collective_compute(
                        "AllReduce", mybir.AluOpType.add,
                        replica_groups=[list(range(cfg.n_cores))],
                        ins=[esum[s * sl_rows:(s + 1) * sl_rows, :].opt()],
                        outs=[esum_red[s * sl_rows:(s + 1) * sl_rows, :].opt()])

            # ---- e_feat = esum_red * (1/edge_deg) -> fp16 table ----
            for et in range(cfg.eb):
                t_in = work.tile([P, OC], dt.float16)
                nc.sync.dma_start(
                    out=t_in[:],
                    in_=(esum if "coll" in ablate else esum_red)[et * P:(et + 1) * P, :])
                ef = work.tile([P, OC], dt.float16)
                nc.vector.tensor_scalar_mul(out=ef[:], in0=t_in[:],
                                            scalar1=re_sb[:, et:et + 1])
                nc.sync.dma_start(out=efeat[et * P:(et + 1) * P, :], in_=ef[:])
            ztb = work.tile([P, OC], dt.float16)
            nc.vector.memset(ztb[:], 0.0)
            nc.sync.dma_start(out=efeat[cfg.ne_pad:cfg.ne_pad + P, :], in_=ztb[:])

            # ---- phase B: vertex means + relu ----
            for g0 in range(0, cfg.vb, cfg.gb_b):
                js = list(range(g0, min(g0 + cfg.gb_b, cfg.vb)))
                t_base = int(off_b[g0])
                tg = int(off_b[js[-1] + 1]) - t_base
                ixt = ipool.tile([P, max_tg * 8], dt.int16)
                nc.sync.dma_start(
                    out=ixt[:, :tg * 8],
                    in_=idxb[:, t_base * 8:(t_base + tg) * 8])
                gt = gpool.tile([P, max_tg, OC], dt.float16)
                o = 0
                if "gb" in ablate:
                    nc.vector.memset(gt[:], 0.0)
                else:
                    for ck in chunks_of(tg * P, cfg.gcap):
                        nc.gpsimd.dma_gather(
                            gt[:, o // P:(o + ck) // P, :], efeat[:, :],
                            ixt[:, o // 16:(o + ck) // 16],
                            ck, ck, OC, single_packet=cfg.gsp)
                        o += ck
                for j in js:
                    w = cfg.bb[j]
                    s = int(off_b[j]) - t_base
                    s_t = spool.tile([P, max_w, P], dt.float16)
                    nc.vector.tensor_tensor(
                        out=s_t[:, :w, :],
                        in0=bcast_free(vloc_sb[:, int(off_b[j]):int(off_b[j]) + w], P),
                        in1=bcast_mid(iota_sb[:, :], w),
                        op=mybir.AluOpType.is_equal)
                    ps = psum.tile([P, OC], dt.float32, space="PSUM")
                    for t in range(w):
                        nc.tensor.matmul(out=ps[:], lhsT=s_t[:, t, :],
                                         rhs=gt[:, s + t, :],
                                         start=(t == 0), stop=(t == w - 1))
                    ot = work.tile([P, OC], dt.float32)
                    nc.vector.tensor_scalar(out=ot[:], in0=ps[:],
                                            scalar1=rv_sb[:, j:j + 1],
                                            scalar2=0.0,
                                            op0=mybir.AluOpType.mult,
                                            op1=mybir.AluOpType.max)
                    nc.sync.dma_start(out=out[j * P:(j + 1) * P, :], in_=ot[:])

    nc.compile()
    return nc


def pack_inputs(cfg: Cfg, X, W, b, v_idx, e_idx):
    """Host-side preprocessing: shard by vertex range, bucket entries (sorted
    by gather target within each bucket), pad per block to the budget, and
    build the per-core input dicts."""
    f16, f32, i16 = np.float16, np.float32, np.int16
    C, VPC, EB, VB = cfg.n_cores, cfg.vpc, cfg.eb, cfg.vb
    NA, NB = cfg.na, cfg.nb
    nv_pad = C * VPC

    off_a = np.concatenate([[0], np.cumsum(cfg.ba)]).astype(np.int64) * P
    off_b = np.concatenate([[0], np.cumsum(cfg.bb)]).astype(np.int64) * P

    v = np.asarray(v_idx).astype(np.int64)
    e = np.asarray(e_idx).astype(np.int64)
    core = v // VPC

    # ----- phase A routing: bucket by (core, edge-block), sorted by vertex -----
    blk = core * EB + e // P
    order = np.lexsort((v, blk))
    cnt = np.bincount(blk, minlength=C * EB)
    assert (cnt.reshape(C, EB) <= np.asarray(cfg.ba) * P).all(), "budget overflow A"
    starts = np.zeros(C * EB, np.int64)
    np.cumsum(cnt[:-1], out=starts[1:])
    ofs = np.arange(len(v), dtype=np.int64) - np.repeat(starts, cnt)
    blk_s = blk[order]
    core_s = blk_s // EB
    eb_s = blk_s % EB
    dest = core_s * NA + off_a[eb_s] + ofs
    idxa_all = np.full(C * NA, VPC, i16)
    idxa_all[dest] = (v[order] - core_s * VPC).astype(i16)
    eloc_all = np.zeros(C * NA, f16)
    eloc_all[dest] = (e[order] % P).astype(f16)

    # ----- phase B routing: bucket by vertex-block, sorted by edge -----
    blkb = v // P                      # == core * VB + local block
    order_b = np.lexsort((e, blkb))
    cntb = np.bincount(blkb, minlength=C * VB)
    assert (cntb.reshape(C, VB) <= np.asarray(cfg.bb) * P).all(), "budget overflow B"
    starts_b = np.zeros(C * VB, np.int64)
    np.cumsum(cntb[:-1], out=starts_b[1:])
    ofs_b = np.arange(len(v), dtype=np.int64) - np.repeat(starts_b, cntb)
    blkb_s = blkb[order_b]
    core_b = blkb_s // VB
    vb_s = blkb_s % VB
    dest_b = core_b * NB + off_b[vb_s] + ofs_b
    idxb_all = np.full(C * NB, cfg.ne_pad, i16)
    idxb_all[dest_b] = e[order_b].astype(i16)
    vloc_all = np.zeros(C * NB, f16)
    vloc_all[dest_b] = (v[order_b] % P).astype(f16)

    # ----- degrees -----
    edeg = np.bincount(e, minlength=cfg.ne_pad).astype(f32)
    re = (1.0 / np.maximum(edeg, 1.0)).astype(f32)
    re_p = np.ascontiguousarray(re.reshape(EB, P).T)
    vdeg = np.bincount(v, minlength=nv_pad).astype(f32)
    rv = (1.0 / np.maximum(vdeg, 1.0)).astype(f32)

    # ----- dense inputs -----
    nv = X.shape[0]
    xt_full = np.zeros((cfg.in_ch, nv_pad), f16)
    xt_full[:, :nv] = np.asarray(X, np.float32).T.astype(f16)
    wt = np.ascontiguousarray(np.asarray(W, np.float32).T.astype(f16))
    bmat = np.tile(np.asarray(b, f32)[None, :], (P, 1))
    iota = np.tile(np.arange(P, dtype=f16)[None, :], (P, 1))

    def wrap16(a):
        # gather index layout: idx i -> [16 partitions, i // 16], replicated x8
        return np.ascontiguousarray(np.tile(a.reshape(-1, 16).T, (P // 16, 1)))

    def pack128(a):
        # per-tile column layout: entry i -> [i % 128, i // 128]
        return np.ascontiguousarray(a.reshape(-1, P).T)

    in_maps = []
    for c in range(C):
        in_maps.append({
            "xt": np.ascontiguousarray(xt_full[:, c * VPC:(c + 1) * VPC]),
            "wt": wt,
            "bmat": bmat,
            "iota": iota,
            "idxa": wrap16(idxa_all[c * NA:(c + 1) * NA]),
            "eloc": pack128(eloc_all[c * NA:(c + 1) * NA]),
            "idxb": wrap16(idxb_all[c * NB:(c + 1) * NB]),
            "vloc": pack128(vloc_all[c * NB:(c + 1) * NB]),
            "re": re_p,
            "rv": np.ascontiguousarray(rv[c * VPC:(c + 1) * VPC].reshape(VB, P).T),
        })
    return in_maps


def make_cfg(v_idx, e_idx, base=REAL):
    """Per-block tile budgets depend on the data; compute them here (max
    over cores so the SPMD program is core-invariant)."""
    v = np.asarray(v_idx).astype(np.int64)
    e = np.asarray(e_idx).astype(np.int64)
    C, VPC = base["n_cores"], base["vpc"]
    eb = base["ne_pad"] // P
    vb = VPC // P
    cnt_a = np.bincount((v // VPC) * eb + e // P,
                        minlength=C * eb).reshape(C, eb)
    ba = tuple(int(x) for x in np.maximum(-(-cnt_a.max(axis=0) // P), 1))
    cnt_b = np.bincount(v // P, minlength=C * vb).reshape(C, vb)
    bb = tuple(int(x) for x in np.maximum(-(-cnt_b.max(axis=0) // P), 1))
    return Cfg(ba=ba, bb=bb, **base)


def run(cfg: Cfg, in_maps, trace=False):
    global LAST_RESULTS
    from concourse.bass_utils import run_bass_kernel_spmd
    key = (cfg.ba, cfg.bb, cfg.gcap, cfg.gsp)
    if key not in _PROG_CACHE:
        _PROG_CACHE[key] = build_program(cfg)
    nc = _PROG_CACHE[key]
    res = run_bass_kernel_spmd(nc, in_maps, core_ids=list(range(cfg.n_cores)),
                               trace=trace)
    LAST_RESULTS = res
    return res


def kernel(X, W, b, v_idx, e_idx, trace=False):
    cfg = make_cfg(v_idx, e_idx)
    in_maps = pack_inputs(cfg, X, W, b, v_idx, e_idx)
    res = run(cfg, in_maps, trace=trace)
    out = np.concatenate([res.results[c]["out"] for c in range(cfg.n_cores)], axis=0)
    return np.ascontiguousarray(out[:N_VERTICES]).astype(np.float32)


# revision 12
# speedup vs baseline: 1.2657x; 1.0141x over previous
# HGNNP hypergraph convolution on 8 Trainium2 NeuronCores (Bass/Tile).
#
# Reference computation:
#   H      = relu(X @ W.T + b)                    [N, 128]
#   e_feat = segment_mean(H[v_idx], e_idx, E)     [E, 128]
#   out    = relu(segment_mean(e_feat[e_idx], v_idx, N))
#
# Strategy (vertex sharding, one kernel launch, in-kernel AllReduce):
#   * Each core owns a contiguous vertex range (VPC rows of X) and computes
#     its H shard with TensorE (X^T is pre-transposed on the host so the
#     contraction dim lands on partitions).
#   * Incidence entries are routed to the core owning their vertex.  Within a
#     core they are bucketed by edge-block (128 edges), sorted by target row,
#     and padded per block to a per-block tile budget (max over cores, so the
#     SPMD program is identical on every core).  A dma_gather pulls H rows
#     for each entry; a per-tile 0/1 selection matrix (is_equal vs an iota
#     row) and a PSUM-accumulated matmul reduce the tile into the block's
#     128 edge rows.
#   * Per-core partial edge sums are AllReduced in 4 slices (the first
#     slices overlap phase A's tail), scaled by 1/edge_degree -> fp16
#     e_feat table.
#   * Phase B mirrors phase A keyed by vertex: gather e_feat rows per entry,
#     selection-matmul into 128-vertex blocks, scale by 1/vertex_degree,
#     relu, write the core's output rows.
#   * Gather payloads are fp16 (256 B/descriptor, the dma_gather minimum);
#     accumulation stays fp32 in PSUM.  The gathers are the only significant
#     device cost (~8 ns/descriptor); per-block budgets trim the padded
#     descriptor count vs a global uniform budget.
from dataclasses import dataclass

import numpy as np

P = 128


def chunks_of(n, cap=9216):
    """Split n idxs into as few chunks as possible, each %128==0 and <=cap.
    Chunk size / single_packet measured performance-neutral on HW from 768
    to 9216 idx/call; big chunks keep Pool-engine desc-gen overhead low."""
    k = -(-n // cap)
    per = -(-n // (k * 128)) * 128
    out = []
    left = n
    while left > 0:
        c = min(per, left)
        out.append(c)
        left -= c
    assert sum(out) == n and all(c % 128 == 0 for c in out)
    return out


@dataclass(frozen=True)
class Cfg:
    n_cores: int
    in_ch: int
    out_ch: int
    vpc: int            # vertices per core (multiple of 128)
    ne_pad: int         # padded edge count (multiple of 128)
    ba: tuple           # per-edge-block tile budgets (len eb)
    bb: tuple           # per-vertex-block tile budgets (len vb)
    gb_a: int           # edge-blocks per gather group in phase A
    gb_b: int           # vertex-blocks per gather group in phase B
    gcap: int = 9216    # max idx per dma_gather call
    gsp: bool = False   # single_packet flag for gather calls

    @property
    def eb(self):
        return self.ne_pad // P

    @property
    def vb(self):
        return self.vpc // P

    @property
    def na(self):
        return sum(self.ba) * P

    @property
    def nb(self):
        return sum(self.bb) * P


# Real problem dimensions.
N_VERTICES = 100000
N_EDGES = 25000
NNZ = 3200000
NV_PAD = 100352           # 8 * 12544
REAL = dict(n_cores=8, in_ch=256, out_ch=128, vpc=12544, ne_pad=25088,
            gb_a=7, gb_b=2)

_PROG_CACHE = {}
LAST_RESULTS = None       # BassKernelResults of the most recent run (for test.py)


def build_program(cfg: Cfg, ablate=frozenset()):
    """Emit the SPMD Bass program (identical on all cores; per-core behavior
    comes entirely from per-core input tensors).

    ablate is a profiling-only hook (see profile_variants.py): "coll" drops
    the AllReduce, "ga"/"gb" replace the phase A/B gathers with memsets.
    The graded path always uses the default empty set."""
    import concourse.bass as bass
    import concourse.mybir as mybir
    import concourse.tile as tile
    from concourse import bacc

    dt = mybir.dt
    OC = cfg.out_ch
    assert cfg.in_ch % P == 0
    KC = cfg.in_ch // P

    # per-block tile offsets (tiles, within a core's stream)
    off_a = np.concatenate([[0], np.cumsum(cfg.ba)]).astype(int)
    off_b = np.concatenate([[0], np.cumsum(cfg.bb)]).astype(int)

    nc = bacc.Bacc("TRN2", target_bir_lowering=False, debug=False,
                   num_devices=cfg.n_cores)

    # ---- I/O ----
    xt = nc.dram_tensor("xt", [cfg.in_ch, cfg.vpc], dt.float16, kind="ExternalInput")
    wt = nc.dram_tensor("wt", [cfg.in_ch, OC], dt.float16, kind="ExternalInput")
    bmat = nc.dram_tensor("bmat", [P, OC], dt.float32, kind="ExternalInput")
    iota = nc.dram_tensor("iota", [P, P], dt.float16, kind="ExternalInput")
    idxa = nc.dram_tensor("idxa", [P, cfg.na // 16], dt.int16, kind="ExternalInput")
    eloc = nc.dram_tensor("eloc", [P, cfg.na // P], dt.float16, kind="ExternalInput")
    idxb = nc.dram_tensor("idxb", [P, cfg.nb // 16], dt.int16, kind="ExternalInput")
    vloc = nc.dram_tensor("vloc", [P, cfg.nb // P], dt.float16, kind="ExternalInput")
    re_p = nc.dram_tensor("re", [P, cfg.eb], dt.float32, kind="ExternalInput")
    rv_p = nc.dram_tensor("rv", [P, cfg.vb], dt.float32, kind="ExternalInput")
    out = nc.dram_tensor("out", [cfg.vpc, OC], dt.float32, kind="ExternalOutput")

    # ---- internal DRAM ----
    hdr = nc.dram_tensor("hdram", [cfg.vpc + P, OC], dt.float16)
    esum = nc.dram_tensor("esum", [cfg.ne_pad + P, OC], dt.float16)
    esum_red = nc.dram_tensor("esum_red", [cfg.ne_pad + P, OC], dt.float16,
                              addr_space="Shared")

    def bcast_free(ap2d, n):
        # [P, C] -> [P, C, n] with the trailing dim broadcast
        return bass.AP(tensor=ap2d.tensor, offset=ap2d.offset,
                       ap=[*ap2d.ap, [0, n]])

    def bcast_mid(ap2d, n):
        # [P, C] -> [P, n, C] with the middle dim broadcast
        return bass.AP(tensor=ap2d.tensor, offset=ap2d.offset,
                       ap=[ap2d.ap[0], [0, n], ap2d.ap[1]])

    max_tg_a = max(sum(cfg.ba[g:g + cfg.gb_a])
                   for g in range(0, cfg.eb, cfg.gb_a))
    max_tg_b = max(sum(cfg.bb[g:g + cfg.gb_b])
                   for g in range(0, cfg.vb, cfg.gb_b))
    max_tg = max(max_tg_a, max_tg_b)
    max_w = max(max(cfg.ba), max(cfg.bb))

    with tile.TileContext(nc) as tc:
        import contextlib
        with contextlib.ExitStack() as ctx:
            const = ctx.enter_context(tc.tile_pool(name="const", bufs=1))
            work = ctx.enter_context(tc.tile_pool(name="work", bufs=3))
            gpool = ctx.enter_context(tc.tile_pool(name="gpool", bufs=2))
            ipool = ctx.enter_context(tc.tile_pool(name="ipool", bufs=2))
            spool = ctx.enter_context(tc.tile_pool(name="spool", bufs=2))
            psum = ctx.enter_context(tc.tile_pool(name="psum", bufs=4, space="PSUM"))

            # ---- constants ----
            xt_sb = const.tile([P, KC, cfg.vpc], dt.float16)
            for k in range(KC):
                nc.sync.dma_start(out=xt_sb[:, k, :], in_=xt[k * P:(k + 1) * P, :])
            wt_sb = const.tile([P, KC, OC], dt.float16)
            for k in range(KC):
                nc.sync.dma_start(out=wt_sb[:, k, :], in_=wt[k * P:(k + 1) * P, :])
            bb_t = const.tile([P, OC], dt.float32)
            nc.sync.dma_start(out=bb_t[:], in_=bmat[:, :])
            iota_sb = const.tile([P, P], dt.float16)
            nc.sync.dma_start(out=iota_sb[:], in_=iota[:, :])
            eloc_sb = const.tile([P, cfg.na // P], dt.float16)
            nc.sync.dma_start(out=eloc_sb[:], in_=eloc[:, :])
            vloc_sb = const.tile([P, cfg.nb // P], dt.float16)
            nc.sync.dma_start(out=vloc_sb[:], in_=vloc[:, :])
            re_sb = const.tile([P, cfg.eb], dt.float32)
            nc.sync.dma_start(out=re_sb[:], in_=re_p[:, :])
            rv_sb = const.tile([P, cfg.vb], dt.float32)
            nc.sync.dma_start(out=rv_sb[:], in_=rv_p[:, :])

            # ---- stage H: H = relu(X @ W.T + b) -> fp16 rows in DRAM ----
            for vt in range(cfg.vb):
                ps = psum.tile([P, OC], dt.float32, space="PSUM")
                for k in range(KC):
                    nc.tensor.matmul(out=ps[:],
                                     lhsT=xt_sb[:, k, vt * P:(vt + 1) * P],
                                     rhs=wt_sb[:, k, :],
                                     start=(k == 0), stop=(k == KC - 1))
                tmp = work.tile([P, OC], dt.float32)
                nc.vector.tensor_add(out=tmp[:], in0=ps[:], in1=bb_t[:])
                h_t = work.tile([P, OC], dt.float16)
                nc.vector.tensor_scalar_max(out=h_t[:], in0=tmp[:], scalar1=0.0)
                nc.sync.dma_start(out=hdr[vt * P:(vt + 1) * P, :], in_=h_t[:])
            zt = work.tile([P, OC], dt.float16)
            nc.vector.memset(zt[:], 0.0)
            nc.sync.dma_start(out=hdr[cfg.vpc:cfg.vpc + P, :], in_=zt[:])

            # ---- phase A: partial edge sums ----
            for g0 in range(0, cfg.eb, cfg.gb_a):
                js = list(range(g0, min(g0 + cfg.gb_a, cfg.eb)))
                t_base = int(off_a[g0])
                tg = int(off_a[js[-1] + 1]) - t_base
                ixt = ipool.tile([P, max_tg * 8], dt.int16)
                nc.sync.dma_start(
                    out=ixt[:, :tg * 8],
                    in_=idxa[:, t_base * 8:(t_base + tg) * 8])
                gt = gpool.tile([P, max_tg, OC], dt.float16)
                o = 0
                if "ga" in ablate:
                    nc.vector.memset(gt[:], 0.0)
                else:
                    for ck in chunks_of(tg * P, cfg.gcap):
                        nc.gpsimd.dma_gather(
                            gt[:, o // P:(o + ck) // P, :], hdr[:, :],
                            ixt[:, o // 16:(o + ck) // 16],
                            ck, ck, OC, single_packet=cfg.gsp)
                        o += ck
                for j in js:
                    w = cfg.ba[j]
                    s = int(off_a[j]) - t_base
                    s_t = spool.tile([P, max_w, P], dt.float16)
                    nc.vector.tensor_tensor(
                        out=s_t[:, :w, :],
                        in0=bcast_free(eloc_sb[:, int(off_a[j]):int(off_a[j]) + w], P),
                        in1=bcast_mid(iota_sb[:, :], w),
                        op=mybir.AluOpType.is_equal)
                    ps = psum.tile([P, OC], dt.float32, space="PSUM")
                    for t in range(w):
                        nc.tensor.matmul(out=ps[:], lhsT=s_t[:, t, :],
                                         rhs=gt[:, s + t, :],
                                         start=(t == 0), stop=(t == w - 1))
                    es = work.tile([P, OC], dt.float16)
                    nc.vector.tensor_scalar_mul(out=es[:], in0=ps[:],
                                                scalar1=re_sb[:, j:j + 1])
                    nc.sync.dma_start(out=esum[j * P:(j + 1) * P, :], in_=es[:])

            # pad block: gathers of the pad token read zeros
            ztb = work.tile([P, OC], dt.float16)
            nc.vector.memset(ztb[:], 0.0)
            nc.sync.dma_start(out=esum[cfg.ne_pad:cfg.ne_pad + P, :], in_=ztb[:])

            # ---- AllReduce pre-scaled edge means (sliced: overlaps phase A) ----
            NSL = 4
            assert cfg.eb % NSL == 0
            sl_rows = (cfg.eb // NSL) * P
            if "coll" not in ablate:
                for s in range(NSL):
                    hi = (s + 1) * sl_rows if s < NSL - 1 else cfg.ne_pad + P
                    nc.gpsimd.collective_compute(
                        "AllReduce", mybir.AluOpType.add,
                        replica_groups=[list(range(cfg.n_cores))],
                        ins=[esum[s * sl_rows:hi, :].opt()],
                        outs=[esum_red[s * sl_rows:hi, :].opt()])

            # ---- phase B: vertex means + relu ----
            for g0 in range(0, cfg.vb, cfg.gb_b):
                js = list(range(g0, min(g0 + cfg.gb_b, cfg.vb)))
                t_base = int(off_b[g0])
                tg = int(off_b[js[-1] + 1]) - t_base
                ixt = ipool.tile([P, max_tg * 8], dt.int16)
                nc.sync.dma_start(
                    out=ixt[:, :tg * 8],
                    in_=idxb[:, t_base * 8:(t_base + tg) * 8])
                gt = gpool.tile([P, max_tg, OC], dt.float16)
                o = 0
                if "gb" in ablate:
                    nc.vector.memset(gt[:], 0.0)
                else:
                    efsrc = esum if "coll" in ablate else esum_red
                    for ck in chunks_of(tg * P, cfg.gcap):
                        nc.gpsimd.dma_gather(
                            gt[:, o // P:(o + ck) // P, :], efsrc[:, :],
                            ixt[:, o // 16:(o + ck) // 16],
                            ck, ck, OC, single_packet=cfg.gsp)
                        o += ck
                for j in js:
                    w = cfg.bb[j]
                    s = int(off_b[j]) - t_base
                    s_t = spool.tile([P, max_w, P], dt.float16)
                    nc.vector.tensor_tensor(
                        out=s_t[:, :w, :],
                        in0=bcast_free(vloc_sb[:, int(off_b[j]):int(off_b[j]) + w], P),
                        in1=bcast_mid(iota_sb[:, :], w),
                        op=mybir.AluOpType.is_equal)
                    ps = psum.tile([P, OC], dt.float32, space="PSUM")
                    for t in range(w):
                        nc.tensor.matmul(out=ps[:], lhsT=s_t[:, t, :],
                                         rhs=gt[:, s + t, :],
                                         start=(t == 0), stop=(t == w - 1))
                    ot = work.tile([P, OC], dt.float32)
                    nc.vector.tensor_scalar(out=ot[:], in0=ps[:],
                                            scalar1=rv_sb[:, j:j + 1],
                                            scalar2=0.0,
                                            op0=mybir.AluOpType.mult,
                                            op1=mybir.AluOpType.max)
                    nc.sync.dma_start(out=out[j * P:(j + 1) * P, :], in_=ot[:])

    nc.compile()
    return nc


def pack_inputs(cfg: Cfg, X, W, b, v_idx, e_idx):
    """Host-side preprocessing: shard by vertex range, bucket entries (sorted
    by gather target within each bucket), pad per block to the budget, and
    build the per-core input dicts."""
    f16, f32, i16 = np.float16, np.float32, np.int16
    C, VPC, EB, VB = cfg.n_cores, cfg.vpc, cfg.eb, cfg.vb
    NA, NB = cfg.na, cfg.nb
    nv_pad = C * VPC

    off_a = np.concatenate([[0], np.cumsum(cfg.ba)]).astype(np.int64) * P
    off_b = np.concatenate([[0], np.cumsum(cfg.bb)]).astype(np.int64) * P

    v = np.asarray(v_idx).astype(np.int64)
    e = np.asarray(e_idx).astype(np.int64)
    core = v // VPC

    # ----- phase A routing: bucket by (core, edge-block), sorted by vertex -----
    blk = core * EB + e // P
    order = np.lexsort((v, blk))
    cnt = np.bincount(blk, minlength=C * EB)
    assert (cnt.reshape(C, EB) <= np.asarray(cfg.ba) * P).all(), "budget overflow A"
    starts = np.zeros(C * EB, np.int64)
    np.cumsum(cnt[:-1], out=starts[1:])
    ofs = np.arange(len(v), dtype=np.int64) - np.repeat(starts, cnt)
    blk_s = blk[order]
    core_s = blk_s // EB
    eb_s = blk_s % EB
    dest = core_s * NA + off_a[eb_s] + ofs
    idxa_all = np.full(C * NA, VPC, i16)
    idxa_all[dest] = (v[order] - core_s * VPC).astype(i16)
    eloc_all = np.zeros(C * NA, f16)
    eloc_all[dest] = (e[order] % P).astype(f16)

    # ----- phase B routing: bucket by vertex-block, sorted by edge -----
    blkb = v // P                      # == core * VB + local block
    order_b = np.lexsort((e, blkb))
    cntb = np.bincount(blkb, minlength=C * VB)
    assert (cntb.reshape(C, VB) <= np.asarray(cfg.bb) * P).all(), "budget overflow B"
    starts_b = np.zeros(C * VB, np.int64)
    np.cumsum(cntb[:-1], out=starts_b[1:])
    ofs_b = np.arange(len(v), dtype=np.int64) - np.repeat(starts_b, cntb)
    blkb_s = blkb[order_b]
    core_b = blkb_s // VB
    vb_s = blkb_s % VB
    dest_b = core_b * NB + off_b[vb_s] + ofs_b
    idxb_all = np.full(C * NB, cfg.ne_pad, i16)
    idxb_all[dest_b] = e[order_b].astype(i16)
    vloc_all = np.zeros(C * NB, f16)
    vloc_all[dest_b] = (v[order_b] % P).astype(f16)

    # ----- degrees -----
    edeg = np.bincount(e, minlength=cfg.ne_pad).astype(f32)
    re = (1.0 / np.maximum(edeg, 1.0)).astype(f32)
    re_p = np.ascontiguousarray(re.reshape(EB, P).T)
    vdeg = np.bincount(v, minlength=nv_pad).astype(f32)
    rv = (1.0 / np.maximum(vdeg, 1.0)).astype(f32)

    # ----- dense inputs -----
    nv = X.shape[0]
    xt_full = np.zeros((cfg.in_ch, nv_pad), f16)
    xt_full[:, :nv] = np.asarray(X, np.float32).T.astype(f16)
    wt = np.ascontiguousarray(np.asarray(W, np.float32).T.astype(f16))
    bmat = np.tile(np.asarray(b, f32)[None, :], (P, 1))
    iota = np.tile(np.arange(P, dtype=f16)[None, :], (P, 1))

    def wrap16(a):
        # gather index layout: idx i -> [16 partitions, i // 16], replicated x8
        return np.ascontiguousarray(np.tile(a.reshape(-1, 16).T, (P // 16, 1)))

    def pack128(a):
        # per-tile column layout: entry i -> [i % 128, i // 128]
        return np.ascontiguousarray(a.reshape(-1, P).T)

    in_maps = []
    for c in range(C):
        in_maps.append({
            "xt": np.ascontiguousarray(xt_full[:, c * VPC:(c + 1) * VPC]),
            "wt": wt,
            "bmat": bmat,
            "iota": iota,
            "idxa": wrap16(idxa_all[c * NA:(c + 1) * NA]),
            "eloc": pack128(eloc_all[c * NA:(c + 1) * NA]),
            "idxb": wrap16(idxb_all[c * NB:(c + 1) * NB]),
            "vloc": pack128(vloc_all[c * NB:(c + 1) * NB]),
            "re": re_p,
            "rv": np.ascontiguousarray(rv[c * VPC:(c + 1) * VPC].reshape(VB, P).T),
        })
    return in_maps


def make_cfg(v_idx, e_idx, base=REAL):
    """Per-block tile budgets depend on the data; compute them here (max
    over cores so the SPMD program is core-invariant)."""
    v = np.asarray(v_idx).astype(np.int64)
    e = np.asarray(e_idx).astype(np.int64)
    C, VPC = base["n_cores"], base["vpc"]
    eb = base["ne_pad"] // P
    vb = VPC // P
    cnt_a = np.bincount((v // VPC) * eb + e // P,
                        minlength=C * eb).reshape(C, eb)
    ba = tuple(int(x) for x in np.maximum(-(-cnt_a.max(axis=0) // P), 1))
    cnt_b = np.bincount(v // P, minlength=C * vb).reshape(C, vb)
    bb = tuple(int(x) for x in np.maximum(-(-cnt_b.max(axis=0) // P), 1))
    return Cfg(ba=ba, bb=bb, **base)


def run(cfg: Cfg, in_maps, trace=False):
    global LAST_RESULTS
    from concourse.bass_utils import run_bass_kernel_spmd
    key = (cfg.ba, cfg.bb, cfg.gcap, cfg.gsp)
    if key not in _PROG_CACHE:
        _PROG_CACHE[key] = build_program(cfg)
    nc = _PROG_CACHE[key]
    res = run_bass_kernel_spmd(nc, in_maps, core_ids=list(range(cfg.n_cores)),
                               trace=trace)
    LAST_RESULTS = res
    return res


def kernel(X, W, b, v_idx, e_idx, trace=False):
    cfg = make_cfg(v_idx, e_idx)
    in_maps = pack_inputs(cfg, X, W, b, v_idx, e_idx)
    res = run(cfg, in_maps, trace=trace)
    out = np.concatenate([res.results[c]["out"] for c in range(cfg.n_cores)], axis=0)
    return np.ascontiguousarray(out[:N_VERTICES]).astype(np.float32)


# revision 14
# speedup vs baseline: 1.2749x; 1.0072x over previous
# HGNNP hypergraph convolution on 8 Trainium2 NeuronCores (Bass/Tile).
#
# Reference computation:
#   H      = relu(X @ W.T + b)                    [N, 128]
#   e_feat = segment_mean(H[v_idx], e_idx, E)     [E, 128]
#   out    = relu(segment_mean(e_feat[e_idx], v_idx, N))
#
# Strategy (vertex sharding, one kernel launch, in-kernel AllReduce):
#   * Each core owns a contiguous vertex range (VPC rows of X) and computes
#     its H shard with TensorE (X^T is pre-transposed on the host so the
#     contraction dim lands on partitions).
#   * Incidence entries are routed to the core owning their vertex.  Within a
#     core they are bucketed by edge-block (128 edges), sorted by target row,
#     and padded per block to a per-block tile budget (max over cores, so the
#     SPMD program is identical on every core).  A dma_gather pulls H rows
#     for each entry; a per-tile 0/1 selection matrix (is_equal vs an iota
#     row) and a PSUM-accumulated matmul reduce the tile into the block's
#     128 edge rows.
#   * Per-core partial edge sums are AllReduced in 4 slices (the first
#     slices overlap phase A's tail), scaled by 1/edge_degree -> fp16
#     e_feat table.
#   * Phase B mirrors phase A keyed by vertex: gather e_feat rows per entry,
#     selection-matmul into 128-vertex blocks, scale by 1/vertex_degree,
#     relu, write the core's output rows.
#   * Gather payloads are fp16 (256 B/descriptor, the dma_gather minimum);
#     accumulation stays fp32 in PSUM.  The gathers are the only significant
#     device cost (~8 ns/descriptor); per-block budgets trim the padded
#     descriptor count vs a global uniform budget.
from dataclasses import dataclass

import numpy as np

P = 128


def chunks_of(n, cap=9216):
    """Split n idxs into as few chunks as possible, each %128==0 and <=cap.
    Chunk size / single_packet measured performance-neutral on HW from 768
    to 9216 idx/call; big chunks keep Pool-engine desc-gen overhead low."""
    k = -(-n // cap)
    per = -(-n // (k * 128)) * 128
    out = []
    left = n
    while left > 0:
        c = min(per, left)
        out.append(c)
        left -= c
    assert sum(out) == n and all(c % 128 == 0 for c in out)
    return out


@dataclass(frozen=True)
class Cfg:
    n_cores: int
    in_ch: int
    out_ch: int
    vpc: int            # vertices per core (multiple of 128)
    ne_pad: int         # padded edge count (multiple of 128)
    ba: tuple           # per-edge-block tile budgets (len eb)
    bb: tuple           # per-vertex-block tile budgets (len vb)
    gb_a: int           # edge-blocks per gather group in phase A
    gb_b: int           # vertex-blocks per gather group in phase B
    gcap: int = 9216    # max idx per dma_gather call
    gsp: bool = False   # single_packet flag for gather calls

    @property
    def eb(self):
        return self.ne_pad // P

    @property
    def vb(self):
        return self.vpc // P

    @property
    def na(self):
        return sum(self.ba) * P

    @property
    def nb(self):
        return sum(self.bb) * P


# Real problem dimensions.
N_VERTICES = 100000
N_EDGES = 25000
NNZ = 3200000
NV_PAD = 100352           # 8 * 12544
REAL = dict(n_cores=8, in_ch=256, out_ch=128, vpc=12544, ne_pad=25088,
            gb_a=7, gb_b=2)

_PROG_CACHE = {}
LAST_RESULTS = None       # BassKernelResults of the most recent run (for test.py)


def build_program(cfg: Cfg, ablate=frozenset()):
    """Emit the SPMD Bass program (identical on all cores; per-core behavior
    comes entirely from per-core input tensors).

    ablate is a profiling-only hook (see profile_variants.py): "coll" drops
    the AllReduce, "ga"/"gb" replace the phase A/B gathers with memsets.
    The graded path always uses the default empty set."""
    import concourse.bass as bass
    import concourse.mybir as mybir
    import concourse.tile as tile
    from concourse import bacc

    dt = mybir.dt
    OC = cfg.out_ch
    assert cfg.in_ch % P == 0
    KC = cfg.in_ch // P

    # per-block tile offsets (tiles, within a core's stream)
    off_a = np.concatenate([[0], np.cumsum(cfg.ba)]).astype(int)
    off_b = np.concatenate([[0], np.cumsum(cfg.bb)]).astype(int)

    nc = bacc.Bacc("TRN2", target_bir_lowering=False, debug=False,
                   num_devices=cfg.n_cores)

    # ---- I/O ----
    xt = nc.dram_tensor("xt", [cfg.in_ch, cfg.vpc], dt.float16, kind="ExternalInput")
    wt = nc.dram_tensor("wt", [cfg.in_ch, OC], dt.float16, kind="ExternalInput")
    bmat = nc.dram_tensor("bmat", [P, OC], dt.float32, kind="ExternalInput")
    iota = nc.dram_tensor("iota", [P, P], dt.float16, kind="ExternalInput")
    idxa = nc.dram_tensor("idxa", [P, cfg.na // 16], dt.int16, kind="ExternalInput")
    eloc = nc.dram_tensor("eloc", [P, cfg.na // P], dt.float16, kind="ExternalInput")
    idxb = nc.dram_tensor("idxb", [P, cfg.nb // 16], dt.int16, kind="ExternalInput")
    vloc = nc.dram_tensor("vloc", [P, cfg.nb // P], dt.float16, kind="ExternalInput")
    re_p = nc.dram_tensor("re", [P, cfg.eb], dt.float32, kind="ExternalInput")
    rv_p = nc.dram_tensor("rv", [P, cfg.vb], dt.float32, kind="ExternalInput")
    out = nc.dram_tensor("out", [cfg.vpc, OC], dt.float32, kind="ExternalOutput")

    # ---- internal DRAM ----
    hdr = nc.dram_tensor("hdram", [cfg.vpc + P, OC], dt.float16)
    esum = nc.dram_tensor("esum", [cfg.ne_pad + P, OC], dt.float16)
    esum_red = nc.dram_tensor("esum_red", [cfg.ne_pad + P, OC], dt.float16,
                              addr_space="Shared")

    def bcast_free(ap2d, n):
        # [P, C] -> [P, C, n] with the trailing dim broadcast
        return bass.AP(tensor=ap2d.tensor, offset=ap2d.offset,
                       ap=[*ap2d.ap, [0, n]])

    def bcast_mid(ap2d, n):
        # [P, C] -> [P, n, C] with the middle dim broadcast
        return bass.AP(tensor=ap2d.tensor, offset=ap2d.offset,
                       ap=[ap2d.ap[0], [0, n], ap2d.ap[1]])

    max_tg_a = max(sum(cfg.ba[g:g + cfg.gb_a])
                   for g in range(0, cfg.eb, cfg.gb_a))
    max_tg_b = max(sum(cfg.bb[g:g + cfg.gb_b])
                   for g in range(0, cfg.vb, cfg.gb_b))
    max_tg = max(max_tg_a, max_tg_b)
    max_w = max(max(cfg.ba), max(cfg.bb))

    with tile.TileContext(nc) as tc:
        import contextlib
        with contextlib.ExitStack() as ctx:
            const = ctx.enter_context(tc.tile_pool(name="const", bufs=1))
            work = ctx.enter_context(tc.tile_pool(name="work", bufs=3))
            gpool = ctx.enter_context(tc.tile_pool(name="gpool", bufs=2))
            ipool = ctx.enter_context(tc.tile_pool(name="ipool", bufs=2))
            spool = ctx.enter_context(tc.tile_pool(name="spool", bufs=2))
            psum = ctx.enter_context(tc.tile_pool(name="psum", bufs=4, space="PSUM"))

            # ---- constants ----
            xt_sb = const.tile([P, KC, cfg.vpc], dt.float16)
            for k in range(KC):
                nc.sync.dma_start(out=xt_sb[:, k, :], in_=xt[k * P:(k + 1) * P, :])
            wt_sb = const.tile([P, KC, OC], dt.float16)
            for k in range(KC):
                nc.sync.dma_start(out=wt_sb[:, k, :], in_=wt[k * P:(k + 1) * P, :])
            bb_t = const.tile([P, OC], dt.float32)
            nc.sync.dma_start(out=bb_t[:], in_=bmat[:, :])
            iota_sb = const.tile([P, P], dt.float16)
            nc.sync.dma_start(out=iota_sb[:], in_=iota[:, :])
            eloc_sb = const.tile([P, cfg.na // P], dt.float16)
            nc.sync.dma_start(out=eloc_sb[:], in_=eloc[:, :])
            vloc_sb = const.tile([P, cfg.nb // P], dt.float16)
            nc.sync.dma_start(out=vloc_sb[:], in_=vloc[:, :])
            re_sb = const.tile([P, cfg.eb], dt.float32)
            nc.sync.dma_start(out=re_sb[:], in_=re_p[:, :])
            rv_sb = const.tile([P, cfg.vb], dt.float32)
            nc.sync.dma_start(out=rv_sb[:], in_=rv_p[:, :])

            # ---- stage H: H = relu(X @ W.T + b) -> fp16 rows in DRAM ----
            for vt in range(cfg.vb):
                ps = psum.tile([P, OC], dt.float32, space="PSUM")
                for k in range(KC):
                    nc.tensor.matmul(out=ps[:],
                                     lhsT=xt_sb[:, k, vt * P:(vt + 1) * P],
                                     rhs=wt_sb[:, k, :],
                                     start=(k == 0), stop=(k == KC - 1))
                tmp = work.tile([P, OC], dt.float32)
                nc.vector.tensor_add(out=tmp[:], in0=ps[:], in1=bb_t[:])
                h_t = work.tile([P, OC], dt.float16)
                nc.vector.tensor_scalar_max(out=h_t[:], in0=tmp[:], scalar1=0.0)
                nc.sync.dma_start(out=hdr[vt * P:(vt + 1) * P, :], in_=h_t[:])
            zt = work.tile([P, OC], dt.float16)
            nc.vector.memset(zt[:], 0.0)
            nc.sync.dma_start(out=hdr[cfg.vpc:cfg.vpc + P, :], in_=zt[:])

            # ---- phase A: partial edge sums ----
            for g0 in range(0, cfg.eb, cfg.gb_a):
                js = list(range(g0, min(g0 + cfg.gb_a, cfg.eb)))
                t_base = int(off_a[g0])
                tg = int(off_a[js[-1] + 1]) - t_base
                ixt = ipool.tile([P, max_tg * 8], dt.int16)
                nc.sync.dma_start(
                    out=ixt[:, :tg * 8],
                    in_=idxa[:, t_base * 8:(t_base + tg) * 8])
                gt = gpool.tile([P, max_tg, OC], dt.float16)
                o = 0
                if "ga" in ablate:
                    nc.vector.memset(gt[:], 0.0)
                else:
                    for ck in chunks_of(tg * P, cfg.gcap):
                        nc.gpsimd.dma_gather(
                            gt[:, o // P:(o + ck) // P, :], hdr[:, :],
                            ixt[:, o // 16:(o + ck) // 16],
                            ck, ck, OC, single_packet=cfg.gsp)
                        o += ck
                for j in js:
                    w = cfg.ba[j]
                    s = int(off_a[j]) - t_base
                    s_t = spool.tile([P, max_w, P], dt.float16)
                    nc.vector.tensor_tensor(
                        out=s_t[:, :w, :],
                        in0=bcast_free(eloc_sb[:, int(off_a[j]):int(off_a[j]) + w], P),
                        in1=bcast_mid(iota_sb[:, :], w),
                        op=mybir.AluOpType.is_equal)
                    ps = psum.tile([P, OC], dt.float32, space="PSUM")
                    for t in range(w):
                        nc.tensor.matmul(out=ps[:], lhsT=s_t[:, t, :],
                                         rhs=gt[:, s + t, :],
                                         start=(t == 0), stop=(t == w - 1))
                    es = work.tile([P, OC], dt.float16)
                    nc.vector.tensor_scalar_mul(out=es[:], in0=ps[:],
                                                scalar1=re_sb[:, j:j + 1])
                    nc.sync.dma_start(out=esum[j * P:(j + 1) * P, :], in_=es[:])

            # pad block: gathers of the pad token read zeros
            ztb = work.tile([P, OC], dt.float16)
            nc.vector.memset(ztb[:], 0.0)
            nc.sync.dma_start(out=esum[cfg.ne_pad:cfg.ne_pad + P, :], in_=ztb[:])

            # ---- AllReduce pre-scaled edge means (sliced: overlaps phase A) ----
            NSL = 4
            assert cfg.eb % NSL == 0
            sl_rows = (cfg.eb // NSL) * P
            if "coll" not in ablate:
                for s in range(NSL):
                    hi = (s + 1) * sl_rows if s < NSL - 1 else cfg.ne_pad + P
                    nc.gpsimd.collective_compute(
                        "AllReduce", mybir.AluOpType.add,
                        replica_groups=[list(range(cfg.n_cores))],
                        ins=[esum[s * sl_rows:hi, :].opt()],
                        outs=[esum_red[s * sl_rows:hi, :].opt()])

            # ---- phase B: vertex means + relu ----
            for g0 in range(0, cfg.vb, cfg.gb_b):
                js = list(range(g0, min(g0 + cfg.gb_b, cfg.vb)))
                t_base = int(off_b[g0])
                tg = int(off_b[js[-1] + 1]) - t_base
                ixt = ipool.tile([P, max_tg * 8], dt.int16)
                nc.sync.dma_start(
                    out=ixt[:, :tg * 8],
                    in_=idxb[:, t_base * 8:(t_base + tg) * 8])
                gt = gpool.tile([P, max_tg, OC], dt.float16)
                o = 0
                if "gb" in ablate:
                    nc.vector.memset(gt[:], 0.0)
                else:
                    efsrc = esum if "coll" in ablate else esum_red
                    for ck in chunks_of(tg * P, cfg.gcap):
                        nc.gpsimd.dma_gather(
                            gt[:, o // P:(o + ck) // P, :], efsrc[:, :],
                            ixt[:, o // 16:(o + ck) // 16],
                            ck, ck, OC, single_packet=cfg.gsp)
                        o += ck
                for j in js:
                    w = cfg.bb[j]
                    s = int(off_b[j]) - t_base
                    s_t = spool.tile([P, max_w, P], dt.float16)
                    nc.vector.tensor_tensor(
                        out=s_t[:, :w, :],
                        in0=bcast_free(vloc_sb[:, int(off_b[j]):int(off_b[j]) + w], P),
                        in1=bcast_mid(iota_sb[:, :], w),
                        op=mybir.AluOpType.is_equal)
                    ps = psum.tile([P, OC], dt.float32, space="PSUM")
                    for t in range(w):
                        nc.tensor.matmul(out=ps[:], lhsT=s_t[:, t, :],
                                         rhs=gt[:, s + t, :],
                                         start=(t == 0), stop=(t == w - 1))
                    ot = work.tile([P, OC], dt.float32)
                    nc.vector.tensor_scalar(out=ot[:], in0=ps[:],
                                            scalar1=rv_sb[:, j:j + 1],
                                            scalar2=0.0,
                                            op0=mybir.AluOpType.mult,
                                            op1=mybir.AluOpType.max)
                    nc.sync.dma_start(out=out[j * P:(j + 1) * P, :], in_=ot[:])

    nc.compile()
    return nc


def pack_inputs(cfg: Cfg, X, W, b, v_idx, e_idx):
    """Host-side preprocessing: shard by vertex range, bucket entries (sorted
    by gather target within each bucket), pad per block to the budget, and
    build the per-core input dicts."""
    f16, f32, i16 = np.float16, np.float32, np.int16
    C, VPC, EB, VB = cfg.n_cores, cfg.vpc, cfg.eb, cfg.vb
    NA, NB = cfg.na, cfg.nb
    nv_pad = C * VPC

    off_a = np.concatenate([[0], np.cumsum(cfg.ba)]).astype(np.int64) * P
    off_b = np.concatenate([[0], np.cumsum(cfg.bb)]).astype(np.int64) * P

    v = np.asarray(v_idx).astype(np.int64)
    e = np.asarray(e_idx).astype(np.int64)
    core = v // VPC

    # ----- phase A routing: bucket by (core, edge-block), sorted by vertex -----
    blk = core * EB + e // P
    order = np.lexsort((v, blk))
    cnt = np.bincount(blk, minlength=C * EB)
    assert (cnt.reshape(C, EB) <= np.asarray(cfg.ba) * P).all(), "budget overflow A"
    starts = np.zeros(C * EB, np.int64)
    np.cumsum(cnt[:-1], out=starts[1:])
    ofs = np.arange(len(v), dtype=np.int64) - np.repeat(starts, cnt)
    blk_s = blk[order]
    core_s = blk_s // EB
    eb_s = blk_s % EB
    dest = core_s * NA + off_a[eb_s] + ofs
    idxa_all = np.full(C * NA, VPC, i16)
    idxa_all[dest] = (v[order] - core_s * VPC).astype(i16)
    eloc_all = np.zeros(C * NA, f16)
    eloc_all[dest] = (e[order] % P).astype(f16)

    # ----- phase B routing: bucket by vertex-block, sorted by edge -----
    blkb = v // P                      # == core * VB + local block
    order_b = np.lexsort((e, blkb))
    cntb = np.bincount(blkb, minlength=C * VB)
    assert (cntb.reshape(C, VB) <= np.asarray(cfg.bb) * P).all(), "budget overflow B"
    starts_b = np.zeros(C * VB, np.int64)
    np.cumsum(cntb[:-1], out=starts_b[1:])
    ofs_b = np.arange(len(v), dtype=np.int64) - np.repeat(starts_b, cntb)
    blkb_s = blkb[order_b]
    core_b = blkb_s // VB
    vb_s = blkb_s % VB
    dest_b = core_b * NB + off_b[vb_s] + ofs_b
    idxb_all = np.full(C * NB, cfg.ne_pad, i16)
    idxb_all[dest_b] = e[order_b].astype(i16)
    vloc_all = np.zeros(C * NB, f16)
    vloc_all[dest_b] = (v[order_b] % P).astype(f16)

    # ----- degrees -----
    edeg = np.bincount(e, minlength=cfg.ne_pad).astype(f32)
    re = (1.0 / np.maximum(edeg, 1.0)).astype(f32)
    re_p = np.ascontiguousarray(re.reshape(EB, P).T)
    vdeg = np.bincount(v, minlength=nv_pad).astype(f32)
    rv = (1.0 / np.maximum(vdeg, 1.0)).astype(f32)

    # ----- dense inputs -----
    nv = X.shape[0]
    xt_full = np.zeros((cfg.in_ch, nv_pad), f16)
    xt_full[:, :nv] = np.asarray(X, np.float32).T.astype(f16)
    wt = np.ascontiguousarray(np.asarray(W, np.float32).T.astype(f16))
    bmat = np.tile(np.asarray(b, f32)[None, :], (P, 1))
    iota = np.tile(np.arange(P, dtype=f16)[None, :], (P, 1))

    def wrap16(a):
        # gather index layout: idx i -> [16 partitions, i // 16], replicated x8
        return np.ascontiguousarray(np.tile(a.reshape(-1, 16).T, (P // 16, 1)))

    def pack128(a):
        # per-tile column layout: entry i -> [i % 128, i // 128]
        return np.ascontiguousarray(a.reshape(-1, P).T)

    in_maps = []
    for c in range(C):
        in_maps.append({
            "xt": np.ascontiguousarray(xt_full[:, c * VPC:(c + 1) * VPC]),
            "wt": wt,
            "bmat": bmat,
            "iota": iota,
            "idxa": wrap16(idxa_all[c * NA:(c + 1) * NA]),
            "eloc": pack128(eloc_all[c * NA:(c + 1) * NA]),
            "idxb": wrap16(idxb_all[c * NB:(c + 1) * NB]),
            "vloc": pack128(vloc_all[c * NB:(c + 1) * NB]),
            "re": re_p,
            "rv": np.ascontiguousarray(rv[c * VPC:(c + 1) * VPC].reshape(VB, P).T),
        })
    return in_maps


def make_cfg(v_idx, e_idx, base=REAL):
    """Per-block tile budgets depend on the data; compute them here (max
    over cores so the SPMD program is core-invariant)."""
    v = np.asarray(v_idx).astype(np.int64)
    e = np.asarray(e_idx).astype(np.int64)
    C, VPC = base["n_cores"], base["vpc"]
    eb = base["ne_pad"] // P
    vb = VPC // P
    cnt_a = np.bincount((v // VPC) * eb + e // P,
                        minlength=C * eb).reshape(C, eb)
    ba = tuple(int(x) for x in np.maximum(-(-cnt_a.max(axis=0) // P), 1))
    cnt_b = np.bincount(v // P, minlength=C * vb).reshape(C, vb)
    bb = tuple(int(x) for x in np.maximum(-(-cnt_b.max(axis=0) // P), 1))
    return Cfg(ba=ba, bb=bb, **base)


def run(cfg: Cfg, in_maps, trace=False):
    global LAST_RESULTS
    from concourse.bass_utils import run_bass_kernel_spmd
    key = (cfg.ba, cfg.bb, cfg.gcap, cfg.gsp)
    if key not in _PROG_CACHE:
        _PROG_CACHE[key] = build_program(cfg)
    nc = _PROG_CACHE[key]
    res = run_bass_kernel_spmd(nc, in_maps, core_ids=list(range(cfg.n_cores)),
                               trace=trace)
    LAST_RESULTS = res
    return res


def kernel(X, W, b, v_idx, e_idx, trace=False):
    cfg = make_cfg(v_idx, e_idx)
    in_maps = pack_inputs(cfg, X, W, b, v_idx, e_idx)
    res = run(cfg, in_maps, trace=trace)
    out = np.concatenate([res.results[c]["out"] for c in range(cfg.n_cores)], axis=0)
    return np.ascontiguousarray(out[:N_VERTICES]).astype(np.float32)


# revision 19
# speedup vs baseline: 1.3052x; 1.0238x over previous
# HGNNP hypergraph convolution on 8 Trainium2 NeuronCores (Bass/Tile).
#
# Reference computation:
#   H      = relu(X @ W.T + b)                    [N, 128]
#   e_feat = segment_mean(H[v_idx], e_idx, E)     [E, 128]
#   out    = relu(segment_mean(e_feat[e_idx], v_idx, N))
#
# Strategy (vertex sharding, one kernel launch, in-kernel AllReduce):
#   * Each core owns a contiguous vertex range (VPC rows of X) and computes
#     its H shard with TensorE (X^T is pre-transposed on the host so the
#     contraction dim lands on partitions).
#   * Incidence entries are routed to the core owning their vertex.  Within a
#     core they are bucketed by edge-block (128 edges), sorted by target row,
#     and padded per block to a per-block tile budget (max over cores, so the
#     SPMD program is identical on every core).  A dma_gather pulls H rows
#     for each entry; a per-tile 0/1 selection matrix (is_equal vs an iota
#     row) and a PSUM-accumulated matmul reduce the tile into the block's
#     128 edge rows.
#   * Per-core partial edge sums are AllReduced in 4 slices (the first
#     slices overlap phase A's tail), scaled by 1/edge_degree -> fp16
#     e_feat table.
#   * Phase B mirrors phase A keyed by vertex: gather e_feat rows per entry,
#     selection-matmul into 128-vertex blocks, scale by 1/vertex_degree,
#     relu, write the core's output rows.
#   * Gather payloads are fp16 (256 B/descriptor, the dma_gather minimum);
#     accumulation stays fp32 in PSUM.  The gathers are the only significant
#     device cost (~8 ns/descriptor); per-block budgets trim the padded
#     descriptor count vs a global uniform budget.
from dataclasses import dataclass

import numpy as np

P = 128


def chunks_of(n, cap=9216):
    """Split n idxs into as few chunks as possible, each %128==0 and <=cap.
    Chunk size / single_packet measured performance-neutral on HW from 768
    to 9216 idx/call; big chunks keep Pool-engine desc-gen overhead low."""
    k = -(-n // cap)
    per = -(-n // (k * 128)) * 128
    out = []
    left = n
    while left > 0:
        c = min(per, left)
        out.append(c)
        left -= c
    assert sum(out) == n and all(c % 128 == 0 for c in out)
    return out


@dataclass(frozen=True)
class Cfg:
    n_cores: int
    in_ch: int
    out_ch: int
    vpc: int            # vertices per core (multiple of 128)
    ne_pad: int         # padded edge count (multiple of 128)
    ba: tuple           # per-edge-block tile budgets (len eb)
    bb: tuple           # per-vertex-block tile budgets (len vb)
    gb_a: int           # edge-blocks per gather group in phase A
    gb_b: int           # vertex-blocks per gather group in phase B
    gcap: int = 9216    # max idx per dma_gather call
    gsp: bool = False   # single_packet flag for gather calls

    @property
    def eb(self):
        return self.ne_pad // P

    @property
    def vb(self):
        return self.vpc // P

    @property
    def na(self):
        return sum(self.ba) * P

    @property
    def nb(self):
        return sum(self.bb) * P


# Real problem dimensions.
N_VERTICES = 100000
N_EDGES = 25000
NNZ = 3200000
NV_PAD = 100352           # 8 * 12544
REAL = dict(n_cores=8, in_ch=256, out_ch=128, vpc=12544, ne_pad=25088,
            gb_a=7, gb_b=2)

_PROG_CACHE = {}
LAST_RESULTS = None       # BassKernelResults of the most recent run (for test.py)


def _slots(code, nbuckets, bspan):
    """Dedup entries sharing a gather target within a bucket into 2-entry
    slots.  code = bucket*bspan + target, one per entry.  Returns
    (order, slot_bucket_counts, per-entry slot offset within its bucket,
    per-entry layer 0/1), all in sorted-entry order."""
    order = np.argsort(code, kind="stable")
    cs = code[order]
    new = np.r_[True, cs[1:] != cs[:-1]]
    gid = np.cumsum(new) - 1
    ucnt = np.bincount(gid)
    grp_start = np.concatenate([[0], np.cumsum(ucnt[:-1])])
    pos = np.arange(len(cs), dtype=np.int64) - grp_start[gid]
    layer = (pos & 1).astype(np.int64)
    subslot = pos >> 1
    slots_g = (ucnt + 1) >> 1
    slot_base = np.concatenate([[0], np.cumsum(slots_g[:-1])])
    slot_id = slot_base[gid] + subslot
    grp_bucket = (cs[new] // bspan).astype(np.int64)
    slot_bucket = np.repeat(grp_bucket, slots_g)
    cnt_slots = np.bincount(slot_bucket, minlength=nbuckets)
    starts_sl = np.concatenate([[0], np.cumsum(cnt_slots[:-1])])
    ofs_sl = (slot_id - starts_sl[(cs // bspan).astype(np.int64)])
    return order, cnt_slots, ofs_sl, layer


def build_program(cfg: Cfg, ablate=frozenset()):
    """Emit the SPMD Bass program (identical on all cores; per-core behavior
    comes entirely from per-core input tensors).

    ablate is a profiling-only hook (see profile_variants.py): "coll" drops
    the AllReduce, "ga"/"gb" replace the phase A/B gathers with memsets.
    The graded path always uses the default empty set."""
    import concourse.bass as bass
    import concourse.mybir as mybir
    import concourse.tile as tile
    from concourse import bacc

    dt = mybir.dt
    OC = cfg.out_ch
    assert cfg.in_ch % P == 0
    KC = cfg.in_ch // P

    # per-block tile offsets (tiles, within a core's stream)
    off_a = np.concatenate([[0], np.cumsum(cfg.ba)]).astype(int)
    off_b = np.concatenate([[0], np.cumsum(cfg.bb)]).astype(int)

    nc = bacc.Bacc("TRN2", target_bir_lowering=False, debug=False,
                   num_devices=cfg.n_cores)

    # ---- I/O ----
    xt = nc.dram_tensor("xt", [cfg.in_ch, cfg.vpc], dt.float16, kind="ExternalInput")
    wt = nc.dram_tensor("wt", [cfg.in_ch, OC], dt.float16, kind="ExternalInput")
    bmat = nc.dram_tensor("bmat", [P, OC], dt.float32, kind="ExternalInput")
    iota = nc.dram_tensor("iota", [P, P], dt.float16, kind="ExternalInput")
    idxa = nc.dram_tensor("idxa", [P, cfg.na // 16], dt.int16, kind="ExternalInput")
    eloc = nc.dram_tensor("eloc", [P, cfg.na // P], dt.float16, kind="ExternalInput")
    eloc2 = nc.dram_tensor("eloc2", [P, cfg.na // P], dt.float16, kind="ExternalInput")
    idxb = nc.dram_tensor("idxb", [P, cfg.nb // 16], dt.int16, kind="ExternalInput")
    vloc = nc.dram_tensor("vloc", [P, cfg.nb // P], dt.float16, kind="ExternalInput")
    vloc2 = nc.dram_tensor("vloc2", [P, cfg.nb // P], dt.float16, kind="ExternalInput")
    re_p = nc.dram_tensor("re", [P, cfg.eb], dt.float32, kind="ExternalInput")
    rv_p = nc.dram_tensor("rv", [P, cfg.vb], dt.float32, kind="ExternalInput")
    out = nc.dram_tensor("out", [cfg.vpc, OC], dt.float32, kind="ExternalOutput")

    # ---- internal DRAM ----
    hdr = nc.dram_tensor("hdram", [cfg.vpc + P, OC], dt.float16)
    esum = nc.dram_tensor("esum", [cfg.ne_pad + P, OC], dt.float16)
    esum_red = nc.dram_tensor("esum_red", [cfg.ne_pad + P, OC], dt.float16,
                              addr_space="Shared")

    def bcast_free(ap2d, n):
        # [P, C] -> [P, C, n] with the trailing dim broadcast
        return bass.AP(tensor=ap2d.tensor, offset=ap2d.offset,
                       ap=[*ap2d.ap, [0, n]])

    def bcast_mid(ap2d, n):
        # [P, C] -> [P, n, C] with the middle dim broadcast
        return bass.AP(tensor=ap2d.tensor, offset=ap2d.offset,
                       ap=[ap2d.ap[0], [0, n], ap2d.ap[1]])

    max_tg_a = max(sum(cfg.ba[g:g + cfg.gb_a])
                   for g in range(0, cfg.eb, cfg.gb_a))
    max_tg_b = max(sum(cfg.bb[g:g + cfg.gb_b])
                   for g in range(0, cfg.vb, cfg.gb_b))
    max_tg = max(max_tg_a, max_tg_b)
    max_w = max(max(cfg.ba), max(cfg.bb))

    with tile.TileContext(nc) as tc:
        import contextlib
        with contextlib.ExitStack() as ctx:
            const = ctx.enter_context(tc.tile_pool(name="const", bufs=1))
            work = ctx.enter_context(tc.tile_pool(name="work", bufs=3))
            gpool = ctx.enter_context(tc.tile_pool(name="gpool", bufs=2))
            ipool = ctx.enter_context(tc.tile_pool(name="ipool", bufs=2))
            spool = ctx.enter_context(tc.tile_pool(name="spool", bufs=2))
            psum = ctx.enter_context(tc.tile_pool(name="psum", bufs=4, space="PSUM"))

            # ---- constants ----
            xt_sb = const.tile([P, KC, cfg.vpc], dt.float16)
            for k in range(KC):
                nc.sync.dma_start(out=xt_sb[:, k, :], in_=xt[k * P:(k + 1) * P, :])
            wt_sb = const.tile([P, KC, OC], dt.float16)
            for k in range(KC):
                nc.sync.dma_start(out=wt_sb[:, k, :], in_=wt[k * P:(k + 1) * P, :])
            bb_t = const.tile([P, OC], dt.float32)
            nc.sync.dma_start(out=bb_t[:], in_=bmat[:, :])
            iota_sb = const.tile([P, P], dt.float16)
            nc.sync.dma_start(out=iota_sb[:], in_=iota[:, :])
            eloc_sb = const.tile([P, cfg.na // P], dt.float16)
            nc.sync.dma_start(out=eloc_sb[:], in_=eloc[:, :])
            eloc2_sb = const.tile([P, cfg.na // P], dt.float16)
            nc.sync.dma_start(out=eloc2_sb[:], in_=eloc2[:, :])
            vloc_sb = const.tile([P, cfg.nb // P], dt.float16)
            nc.sync.dma_start(out=vloc_sb[:], in_=vloc[:, :])
            vloc2_sb = const.tile([P, cfg.nb // P], dt.float16)
            nc.sync.dma_start(out=vloc2_sb[:], in_=vloc2[:, :])
            re_sb = const.tile([P, cfg.eb], dt.float32)
            nc.sync.dma_start(out=re_sb[:], in_=re_p[:, :])
            rv_sb = const.tile([P, cfg.vb], dt.float32)
            nc.sync.dma_start(out=rv_sb[:], in_=rv_p[:, :])

            # ---- stage H: H = relu(X @ W.T + b) -> fp16 rows in DRAM ----
            for vt in range(cfg.vb):
                ps = psum.tile([P, OC], dt.float32, space="PSUM")
                for k in range(KC):
                    nc.tensor.matmul(out=ps[:],
                                     lhsT=xt_sb[:, k, vt * P:(vt + 1) * P],
                                     rhs=wt_sb[:, k, :],
                                     start=(k == 0), stop=(k == KC - 1))
                tmp = work.tile([P, OC], dt.float32)
                nc.vector.tensor_add(out=tmp[:], in0=ps[:], in1=bb_t[:])
                h_t = work.tile([P, OC], dt.float16)
                nc.scalar.activation(out=h_t[:], in_=tmp[:],
                                     func=mybir.ActivationFunctionType.Relu)
                nc.sync.dma_start(out=hdr[vt * P:(vt + 1) * P, :], in_=h_t[:])
            zt = work.tile([P, OC], dt.float16)
            nc.vector.memset(zt[:], 0.0)
            nc.sync.dma_start(out=hdr[cfg.vpc:cfg.vpc + P, :], in_=zt[:])

            # ---- phase A: partial edge sums ----
            for g0 in range(0, cfg.eb, cfg.gb_a):
                js = list(range(g0, min(g0 + cfg.gb_a, cfg.eb)))
                t_base = int(off_a[g0])
                tg = int(off_a[js[-1] + 1]) - t_base
                ixt = ipool.tile([P, max_tg * 8], dt.int16)
                nc.sync.dma_start(
                    out=ixt[:, :tg * 8],
                    in_=idxa[:, t_base * 8:(t_base + tg) * 8])
                gt = gpool.tile([P, max_tg, OC], dt.float16)
                o = 0
                if "ga" in ablate:
                    nc.vector.memset(gt[:], 0.0)
                else:
                    for ck in chunks_of(tg * P, cfg.gcap):
                        nc.gpsimd.dma_gather(
                            gt[:, o // P:(o + ck) // P, :], hdr[:, :],
                            ixt[:, o // 16:(o + ck) // 16],
                            ck, ck, OC, single_packet=cfg.gsp)
                        o += ck
                for j in js:
                    w = cfg.ba[j]
                    s = int(off_a[j]) - t_base
                    s_t = spool.tile([P, max_w, P], dt.float16)
                    nc.vector.tensor_tensor(
                        out=s_t[:, :w, :],
                        in0=bcast_free(eloc_sb[:, int(off_a[j]):int(off_a[j]) + w], P),
                        in1=bcast_mid(iota_sb[:, :], w),
                        op=mybir.AluOpType.is_equal)
                    s_u = spool.tile([P, max_w, P], dt.float16)
                    nc.vector.tensor_tensor(
                        out=s_u[:, :w, :],
                        in0=bcast_free(eloc2_sb[:, int(off_a[j]):int(off_a[j]) + w], P),
                        in1=bcast_mid(iota_sb[:, :], w),
                        op=mybir.AluOpType.is_equal)
                    nc.vector.tensor_add(out=s_t[:, :w, :], in0=s_t[:, :w, :],
                                         in1=s_u[:, :w, :])
                    ps = psum.tile([P, OC], dt.float32, space="PSUM")
                    for t in range(w):
                        nc.tensor.matmul(out=ps[:], lhsT=s_t[:, t, :],
                                         rhs=gt[:, s + t, :],
                                         start=(t == 0), stop=(t == w - 1))
                    es = work.tile([P, OC], dt.float16)
                    nc.scalar.activation(out=es[:], in_=ps[:],
                                         func=mybir.ActivationFunctionType.Identity,
                                         scale=re_sb[:, j:j + 1])
                    nc.sync.dma_start(out=esum[j * P:(j + 1) * P, :], in_=es[:])

            # pad block: gathers of the pad token read zeros
            ztb = work.tile([P, OC], dt.float16)
            nc.vector.memset(ztb[:], 0.0)
            nc.sync.dma_start(out=esum[cfg.ne_pad:cfg.ne_pad + P, :], in_=ztb[:])

            # ---- AllReduce pre-scaled edge means (sliced: overlaps phase A) ----
            NSL = 4
            assert cfg.eb % NSL == 0
            sl_rows = (cfg.eb // NSL) * P
            if "coll" not in ablate:
                for s in range(NSL):
                    hi = (s + 1) * sl_rows if s < NSL - 1 else cfg.ne_pad + P
                    nc.gpsimd.collective_compute(
                        "AllReduce", mybir.AluOpType.add,
                        replica_groups=[list(range(cfg.n_cores))],
                        ins=[esum[s * sl_rows:hi, :].opt()],
                        outs=[esum_red[s * sl_rows:hi, :].opt()])

            # ---- phase B: vertex means + relu ----
            for g0 in range(0, cfg.vb, cfg.gb_b):
                js = list(range(g0, min(g0 + cfg.gb_b, cfg.vb)))
                t_base = int(off_b[g0])
                tg = int(off_b[js[-1] + 1]) - t_base
                ixt = ipool.tile([P, max_tg * 8], dt.int16)
                nc.sync.dma_start(
                    out=ixt[:, :tg * 8],
                    in_=idxb[:, t_base * 8:(t_base + tg) * 8])
                gt = gpool.tile([P, max_tg, OC], dt.float16)
                o = 0
                if "gb" in ablate:
                    nc.vector.memset(gt[:], 0.0)
                else:
                    efsrc = esum if "coll" in ablate else esum_red
                    for ck in chunks_of(tg * P, cfg.gcap):
                        nc.gpsimd.dma_gather(
                            gt[:, o // P:(o + ck) // P, :], efsrc[:, :],
                            ixt[:, o // 16:(o + ck) // 16],
                            ck, ck, OC, single_packet=cfg.gsp)
                        o += ck
                for j in js:
                    w = cfg.bb[j]
                    s = int(off_b[j]) - t_base
                    s_t = spool.tile([P, max_w, P], dt.float16)
                    nc.vector.tensor_tensor(
                        out=s_t[:, :w, :],
                        in0=bcast_free(vloc_sb[:, int(off_b[j]):int(off_b[j]) + w], P),
                        in1=bcast_mid(iota_sb[:, :], w),
                        op=mybir.AluOpType.is_equal)
                    s_u = spool.tile([P, max_w, P], dt.float16)
                    nc.vector.tensor_tensor(
                        out=s_u[:, :w, :],
                        in0=bcast_free(vloc2_sb[:, int(off_b[j]):int(off_b[j]) + w], P),
                        in1=bcast_mid(iota_sb[:, :], w),
                        op=mybir.AluOpType.is_equal)
                    nc.vector.tensor_add(out=s_t[:, :w, :], in0=s_t[:, :w, :],
                                         in1=s_u[:, :w, :])
                    ps = psum.tile([P, OC], dt.float32, space="PSUM")
                    for t in range(w):
                        nc.tensor.matmul(out=ps[:], lhsT=s_t[:, t, :],
                                         rhs=gt[:, s + t, :],
                                         start=(t == 0), stop=(t == w - 1))
                    ot = work.tile([P, OC], dt.float32)
                    nc.scalar.activation(out=ot[:], in_=ps[:],
                                         func=mybir.ActivationFunctionType.Relu,
                                         scale=rv_sb[:, j:j + 1])
                    nc.sync.dma_start(out=out[j * P:(j + 1) * P, :], in_=ot[:])

    nc.compile()
    return nc


def pack_inputs(cfg: Cfg, X, W, b, v_idx, e_idx):
    """Host-side preprocessing: shard by vertex range, bucket entries (sorted
    by gather target within each bucket), pad per block to the budget, and
    build the per-core input dicts."""
    f16, f32, i16 = np.float16, np.float32, np.int16
    C, VPC, EB, VB = cfg.n_cores, cfg.vpc, cfg.eb, cfg.vb
    NA, NB = cfg.na, cfg.nb
    nv_pad = C * VPC

    off_a = np.concatenate([[0], np.cumsum(cfg.ba)]).astype(np.int64) * P
    off_b = np.concatenate([[0], np.cumsum(cfg.bb)]).astype(np.int64) * P

    v = np.asarray(v_idx).astype(np.int64)
    e = np.asarray(e_idx).astype(np.int64)
    core = v // VPC

    # ----- phase A routing: slots keyed by (core, edge-block, vertex) -----
    blk = core * EB + e // P
    NVG = C * VPC
    order, cnt_sl, ofs_sl, layer = _slots(blk * np.int64(NVG) + v, C * EB, NVG)
    assert (cnt_sl.reshape(C, EB) <= np.asarray(cfg.ba) * P).all(), "budget overflow A"
    blk_s = blk[order]
    core_s = blk_s // EB
    eb_s = blk_s % EB
    dest = core_s * NA + off_a[eb_s] + ofs_sl
    idxa_all = np.full(C * NA, VPC, i16)
    idxa_all[dest] = (v[order] - core_s * VPC).astype(i16)
    eloc_all = np.zeros(C * NA, f16)
    eloc2_all = np.full(C * NA, 255.0, f16)
    l0, l1 = layer == 0, layer == 1
    eloc_all[dest[l0]] = (e[order][l0] % P).astype(f16)
    eloc2_all[dest[l1]] = (e[order][l1] % P).astype(f16)

    # ----- phase B routing: slots keyed by (vertex-block, edge) -----
    blkb = v // P                      # == core * VB + local block
    NEG = cfg.ne_pad
    order_b, cnt_slb, ofs_slb, layer_b = _slots(
        blkb * np.int64(NEG) + e, C * VB, NEG)
    assert (cnt_slb.reshape(C, VB) <= np.asarray(cfg.bb) * P).all(), "budget overflow B"
    blkb_s = blkb[order_b]
    core_b = blkb_s // VB
    vb_s = blkb_s % VB
    dest_b = core_b * NB + off_b[vb_s] + ofs_slb
    idxb_all = np.full(C * NB, cfg.ne_pad, i16)
    idxb_all[dest_b] = e[order_b].astype(i16)
    vloc_all = np.zeros(C * NB, f16)
    vloc2_all = np.full(C * NB, 255.0, f16)
    m0, m1 = layer_b == 0, layer_b == 1
    vloc_all[dest_b[m0]] = (v[order_b][m0] % P).astype(f16)
    vloc2_all[dest_b[m1]] = (v[order_b][m1] % P).astype(f16)

    # ----- degrees -----
    edeg = np.bincount(e, minlength=cfg.ne_pad).astype(f32)
    re = (1.0 / np.maximum(edeg, 1.0)).astype(f32)
    re_p = np.ascontiguousarray(re.reshape(EB, P).T)
    vdeg = np.bincount(v, minlength=nv_pad).astype(f32)
    rv = (1.0 / np.maximum(vdeg, 1.0)).astype(f32)

    # ----- dense inputs -----
    nv = X.shape[0]
    xt_full = np.zeros((cfg.in_ch, nv_pad), f16)
    xt_full[:, :nv] = np.asarray(X, np.float32).T.astype(f16)
    wt = np.ascontiguousarray(np.asarray(W, np.float32).T.astype(f16))
    bmat = np.tile(np.asarray(b, f32)[None, :], (P, 1))
    iota = np.tile(np.arange(P, dtype=f16)[None, :], (P, 1))

    def wrap16(a):
        # gather index layout: idx i -> [16 partitions, i // 16], replicated x8
        return np.ascontiguousarray(np.tile(a.reshape(-1, 16).T, (P // 16, 1)))

    def pack128(a):
        # per-tile column layout: entry i -> [i % 128, i // 128]
        return np.ascontiguousarray(a.reshape(-1, P).T)

    in_maps = []
    for c in range(C):
        in_maps.append({
            "xt": np.ascontiguousarray(xt_full[:, c * VPC:(c + 1) * VPC]),
            "wt": wt,
            "bmat": bmat,
            "iota": iota,
            "idxa": wrap16(idxa_all[c * NA:(c + 1) * NA]),
            "eloc": pack128(eloc_all[c * NA:(c + 1) * NA]),
            "eloc2": pack128(eloc2_all[c * NA:(c + 1) * NA]),
            "idxb": wrap16(idxb_all[c * NB:(c + 1) * NB]),
            "vloc": pack128(vloc_all[c * NB:(c + 1) * NB]),
            "vloc2": pack128(vloc2_all[c * NB:(c + 1) * NB]),
            "re": re_p,
            "rv": np.ascontiguousarray(rv[c * VPC:(c + 1) * VPC].reshape(VB, P).T),
        })
    return in_maps


def make_cfg(v_idx, e_idx, base=REAL):
    """Per-block tile budgets depend on the data; compute them here (max
    over cores so the SPMD program is core-invariant)."""
    v = np.asarray(v_idx).astype(np.int64)
    e = np.asarray(e_idx).astype(np.int64)
    C, VPC = base["n_cores"], base["vpc"]
    eb = base["ne_pad"] // P
    vb = VPC // P
    blk = (v // VPC) * eb + e // P
    _, cnt_a, _, _ = _slots(blk * np.int64(C * VPC) + v, C * eb, C * VPC)
    ba = tuple(int(x) for x in
               np.maximum(-(-cnt_a.reshape(C, eb).max(axis=0) // P), 1))
    ne_pad = base["ne_pad"]
    _, cnt_b, _, _ = _slots((v // P) * np.int64(ne_pad) + e, C * vb, ne_pad)
    bb = tuple(int(x) for x in
               np.maximum(-(-cnt_b.reshape(C, vb).max(axis=0) // P), 1))
    return Cfg(ba=ba, bb=bb, **base)


def run(cfg: Cfg, in_maps, trace=False):
    global LAST_RESULTS
    from concourse.bass_utils import run_bass_kernel_spmd
    key = (cfg.ba, cfg.bb, cfg.gcap, cfg.gsp)
    if key not in _PROG_CACHE:
        _PROG_CACHE[key] = build_program(cfg)
    nc = _PROG_CACHE[key]
    res = run_bass_kernel_spmd(nc, in_maps, core_ids=list(range(cfg.n_cores)),
                               trace=trace)
    LAST_RESULTS = res
    return res


def kernel(X, W, b, v_idx, e_idx, trace=False):
    cfg = make_cfg(v_idx, e_idx)
    in_maps = pack_inputs(cfg, X, W, b, v_idx, e_idx)
    res = run(cfg, in_maps, trace=trace)
    out = np.concatenate([res.results[c]["out"] for c in range(cfg.n_cores)], axis=0)
    return np.ascontiguousarray(out[:N_VERTICES]).astype(np.float32)
